# revision 31
# baseline (speedup 1.0000x reference)
"""Trainium2 Bass kernel for Angles2Backbone (NeRF chain forward).

Full inputs: input [256,3,512] f32, param [6] f32, angles_length [256] i32.
Output: [256, 4608] f32  (coords of 1536 backbone atoms x 3, masked).

Sharding: pure data parallel over batch - 32 proteins per core x 8 cores.

Per-core algorithm (v3: fp16 scan machinery, mod-4 blocked residue layout):
  - 128 partitions = (quarter q)*32 + protein b; each row owns 128 residues.
  - Residue r of a quarter lives at column sigma(r) = 32*(r%4) + r//4
    (host-side permutation), so every pair/quad/expansion op reads and
    writes stride-1 column blocks -> DVE 2x fp16 mode throughout.
  - Trig via sin LUT at a/8, a/4 + double-angle chains (fp16).
  - Pre-pass builds per-residue rotation Rres (fp16) from scalar-folded
    bilinear terms; v-vectors (per-atom translations) in fp16.
  - pair (mod-4 blocks) -> P2, quad -> P4, Hillis-Steele over 32 quads.
  - Vector expansions Wodd/W2 written straight into a fused source tensor
    US so the superquad expansion is 5 big ops against a replicated,
    shifted prefix tensor L4.
  - Positions: blocked u -> reorder copy -> masked tensor_tensor_scan
    seeded with F^T.Pinc (cross-quarter fixup folded into the scan),
    then frame rotation F and store.
  - Cross-partition moves via PE matmuls only (no SBUF-SBUF DMA).
"""

import sys

sys.path.insert(0, "/opt/trn_rl_repo")

import numpy as np
import concourse.bass as bass
import concourse.bacc as bacc
import concourse.mybir as mybir
from concourse import tile
from concourse.bass_utils import run_bass_kernel_spmd

F32 = mybir.dt.float32
F16 = mybir.dt.float16
I32 = mybir.dt.int32
AF = mybir.ActivationFunctionType
OP = mybir.AluOpType

NCORES = 8
BPC = 32          # proteins per core
L = 512           # residues per protein
QN = 4            # chain quarters per protein (partition groups)
W = 384           # atoms per quarter
NR = 128          # residues per quarter
NB = 32           # columns per mod-4 block
PI = float(np.pi)

_CACHE = {}


# const block layouts (per partition)
C16_SELBQ = 0          # [512] selbq one-hots (fp16)
C16_L4 = 512           # [1152] L4 zero-init (fp16)
C16_ONESR = 1664       # [128] ones (fp16)
C16_N = 1792
C32_SELCUM = 0         # [128] cumulative masked selector (f32)
C32_SELBT = 128        # [128] selbT rows 0:32 (f32)
C32_JPLANE = 256       # [384] atom index (f32)
C32_N = 640


def _build_graph():
    nc = bacc.Bacc("TRN2", target_bir_lowering=False, debug=False,
                   num_devices=NCORES)
    inp = nc.dram_tensor("input", [QN * BPC, 3 * NR], F32,
                         kind="ExternalInput").ap()
    par = nc.dram_tensor("param", [6], F32, kind="ExternalInput").ap()
    alen = nc.dram_tensor("angles_length", [BPC], I32,
                          kind="ExternalInput").ap()
    c16 = nc.dram_tensor("c16", [128, C16_N], F16,
                         kind="ExternalInput").ap()
    c32 = nc.dram_tensor("c32", [128, C32_N], F32,
                         kind="ExternalInput").ap()
    out = nc.dram_tensor("out", [QN * BPC, 3 * W], F16,
                         kind="ExternalOutput").ap()
    with tile.TileContext(nc) as tc:
        _emit(nc, tc, inp, par, alen, c16, c32, out)
    nc.compile()
    return nc


def _ap(base_ap, off, dims):
    return bass.AP(base_ap.tensor, base_ap.offset + off,
                   [list(base_ap.ap[0])] + [list(d) for d in dims])


def _emit(nc, tc, inp, par, alen, c16, c32, out):
    import contextlib
    ctx = contextlib.ExitStack()
    with ctx:
        main = ctx.enter_context(tc.tile_pool(name="main", bufs=1))
        psum = ctx.enter_context(tc.tile_pool(name="psum", bufs=1,
                                              space="PSUM"))

        # ---------------- tiles ----------------
        alpha = main.tile([128, 3 * NR], F32, tag="alpha")
        ca = main.tile([128, 3 * NR], F16, tag="ca")
        sa = main.tile([128, 3 * NR], F16, tag="sa")
        # trig scratch (magic-number range reduction)
        tsq = main.tile([128, 3 * NR], F32, tag="tsq")   # squares scratch
        ts2 = main.tile([128, 3 * NR], F32, tag="ts2")   # scratch
        tmagic = main.tile([128, 3 * NR], F32, tag="tmagic")
        thalf = main.tile([128, 3 * NR], F32, tag="thalf")

        PP = main.tile([128, 4 * NR], F16, tag="PP")
        C1 = main.tile([128, 9 * NR], F16, tag="C1")
        QQ = main.tile([128, 6 * NR], F16, tag="QQ")
        RA = main.tile([128, 9 * NR], F16, tag="RA")     # Rres fp16
        vm = main.tile([128, 9 * NR], F16, tag="vm")
        P2 = main.tile([128, 9 * 2 * NB], F16, tag="P2")
        P2e2 = main.tile([128, 9 * 2 * NB], F16, tag="P2e2")
        P4A = main.tile([128, 9 * NB], F16, tag="P4A")
        P4B = main.tile([128, 9 * NB], F16, tag="P4B")
        US = main.tile([128, 9 * 4 * NB], F16, tag="US")
        WS = main.tile([128, 9 * 2 * NB], F16, tag="WS")
        T9a = main.tile([128, 9 * 4 * NB], F16, tag="T9a")
        T9b = main.tile([128, 9 * 4 * NB], F16, tag="T9b")
        T9c = main.tile([128, 9 * 4 * NB], F16, tag="T9c")
        T9w1 = main.tile([128, 9 * 2 * NB], F16, tag="T9w1")
        T9w2 = main.tile([128, 9 * 2 * NB], F16, tag="T9w2")
        T9w3 = main.tile([128, 9 * 2 * NB], F16, tag="T9w3")
        T9x = main.tile([128, 3 * W], F16, tag="T9x")
        T9y = main.tile([128, 3 * W], F16, tag="T9y")
        T9z = main.tile([128, 3 * W], F16, tag="T9z")
        T9af = main.tile([128, 18 * NB], F32, tag="T9af")
        T9bf = main.tile([128, 18 * NB], F32, tag="T9bf")
        T9cf = main.tile([128, 18 * NB], F32, tag="T9cf")
        Uloc = main.tile([128, 3 * W], F16, tag="Uloc")  # blocked u
        Uord = main.tile([128, 3 * W], F16, tag="Uord")  # ordered u
        Pall = main.tile([128, 3 * W], F16, tag="Pall")  # scanned+seeded out

        maskp = main.tile([128, W], F16, tag="maskp")
        thr = main.tile([128, 1], F32, tag="thr")
        Lsb = main.tile([BPC, 1], I32, tag="Lsb")
        Lf = main.tile([BPC, 1], F32, tag="Lf")
        Psb = main.tile([1, 6], F32, tag="Psb")
        kv = main.tile([1, 3], F32, tag="kv")
        Rv = main.tile([1, 3], F32, tag="Rv")
        NSC = 24
        vecs = main.tile([1, NSC], F32, tag="vecs")
        Vb = main.tile([128, NSC], F32, tag="Vb")
        zb1 = main.tile([1, 1], F32, tag="zb1")
        zb128 = main.tile([128, 1], F32, tag="zb128")
        warm = main.tile([1, 1], F32, tag="warm")

        # DMA'd constants (selbq one-hots, L4 zero init, ones, SELcum,
        # selbT, jplane)
        CON16 = main.tile([128, C16_N], F16, tag="CON16")
        CON32 = main.tile([128, C32_N], F32, tag="CON32")
        L4v = CON16[:, C16_L4:C16_L4 + 9 * 4 * NB]
        onesr = CON16[:, C16_ONESR:C16_ONESR + NR]
        SELcum = CON32[:, C32_SELCUM:C32_SELCUM + 128]
        selbTv = CON32[0:BPC, C32_SELBT:C32_SELBT + 128]
        jplane = CON32[:, C32_JPLANE:C32_JPLANE + W]

        # cross-quarter fixup (redundantly on all 128 rows, f32)
        Estack = main.tile([128, 36], F32, tag="Estack")
        Fstack = main.tile([128, 27], F32, tag="Fstack")
        Fbc = main.tile([128, 9], F32, tag="Fbc")
        mt0 = main.tile([128, 9], F32, tag="mt0")
        mt1 = main.tile([128, 9], F32, tag="mt1")
        # tail: local sums -> world quarter increments
        Tloc = main.tile([128, 3], F32, tag="Tloc")
        S9 = main.tile([128, 9], F32, tag="S9")
        S3 = main.tile([128, 3], F32, tag="S3")
        FtPbS = main.tile([128, 3], F32, tag="FtPbS")

        PSg = psum.tile([128, 36], F32, tag="PSg")
        PSc = psum.tile([128, 3], F32, tag="PSc")
        PSL = psum.tile([128, 1], F32, tag="PSL")

        V = nc.vector
        G = nc.gpsimd
        A = nc.scalar
        STT = nc.vector.scalar_tensor_tensor
        TS = nc.vector.tensor_scalar
        GTS = nc.gpsimd.tensor_scalar

        # ============ Phase A: DMAs + ACT warmup + setup ============
        nc.sync.dma_start(alpha[:], inp[:])
        nc.sync.dma_start(Psb[:], par[:])
        nc.sync.dma_start(Lsb[:], alen[:])
        nc.gpsimd.dma_start(CON16[:], c16[:])
        nc.gpsimd.dma_start(CON32[:], c32[:])
        V.memset(zb1[:], 0.0)
        V.memset(zb128[:], 0.0)
        # trigger the Sin table load immediately (Copy set loads after sins)
        A.activation(warm[:], zb1[:], AF.Sin, bias=zb1[:])

        # ============ Phase B: trig (2^23 magic range reduction + Sin) ====
        # ar = alpha - 2pi*round(alpha/2pi) in [-pi, pi]; sa = sin(ar);
        # ca = 1 - 2*sin(ar/2)^2. 1.5*2^23 keeps the rounding add in the
        # [2^23, 2^24) binade (ulp=1) for negative args too.
        # Per type block (N, CA, C) to pipeline the serial chain. Emitted
        # BEFORE the param-scalar block: DVE dispatch is in-order and the
        # param copies wait on the (later) param DMA sem.
        MAGIC = float(3 * 2 ** 22)

        def trig_dve(t):
            bs = slice(t * NR, (t + 1) * NR)
            V.tensor_scalar(tmagic[:, bs], alpha[:, bs], 1.0 / (2 * PI),
                            MAGIC, op0=OP.mult, op1=OP.add)
            V.tensor_scalar(tmagic[:, bs], tmagic[:, bs], MAGIC, None,
                            op0=OP.subtract)
            STT(ts2[:, bs], tmagic[:, bs], -2 * PI, alpha[:, bs],
                op0=OP.mult, op1=OP.add)

        def trig_sin(t):
            bs = slice(t * NR, (t + 1) * NR)
            A.activation(sa[:, bs], ts2[:, bs], AF.Sin, bias=zb128[:])
            A.activation(thalf[:, bs], ts2[:, bs], AF.Sin, bias=zb128[:],
                         scale=0.5)

        def trig_cos(t):
            bs = slice(t * NR, (t + 1) * NR)
            G.tensor_mul(tsq[:, bs], thalf[:, bs], thalf[:, bs])
            G.tensor_scalar(ca[:, bs], tsq[:, bs], -2.0, 1.0,
                            op0=OP.mult, op1=OP.add)

        trig_dve(0)
        trig_sin(0)
        trig_dve(1)

        # param scalars (wait on the param DMA; emitted between trig blocks)
        for t, idx in enumerate((5, 1, 3)):   # kappa: CA_C_N, C_N_CA, N_CA_C
            V.tensor_copy(kv[0:1, t:t + 1], Psb[0:1, idx:idx + 1])
        for t, idx in enumerate((4, 0, 2)):   # R: R_C_N, R_N_CA, R_CA_C
            V.tensor_copy(Rv[0:1, t:t + 1], Psb[0:1, idx:idx + 1])
        sk3 = main.tile([1, 3], F32, tag="sk3")
        ck3 = main.tile([1, 3], F32, tag="ck3")
        kvr = main.tile([1, 3], F32, tag="kvr")
        A.activation(sk3[:], kv[0:1, 0:3], AF.Sin, bias=zb1[:])
        A.activation(kvr[:], kv[0:1, 0:3], AF.Sin, bias=zb1[:], scale=0.5)
        trig_sin(1)
        trig_cos(0)
        trig_dve(2)
        trig_sin(2)
        trig_cos(1)
        trig_cos(2)
        A.copy(warm[:], zb1[:])     # Copy-set LUT load after the last Sin

        V.tensor_mul(kvr[:], kvr[:], kvr[:])
        V.tensor_scalar(ck3[:], kvr[:], -2.0, 1.0, op0=OP.mult, op1=OP.add)

        # scalar slots in vecs[1, NSC]:
        # 0:ckN 1:skN 2:ckA 3:skA 4:ckC 5:skC
        # 6:ckNckA 7:ckNskA 8:skNckA 9:skNskA
        # 10:nckN 11:nckA 12:nckC 13:nskA 14:nckNskA 15:nskNckA
        # 16:RNckN 17:RNskN 18:RCA 19:RC
        def vc(i):
            return vecs[0:1, i:i + 1]

        # interleave ck/sk into slots 0..5
        V.tensor_copy(_ap(vecs[:], 0, [[2, 3]]), ck3[0:1, 0:3])
        V.tensor_copy(_ap(vecs[:], 1, [[2, 3]]), sk3[0:1, 0:3])
        # outer product (ckN,skN) x (ckA,skA) -> slots 6..9
        V.tensor_mul(_ap(vecs[:], 6, [[2, 2], [1, 2]]),
                     _ap(vecs[:], 0, [[1, 2], [0, 2]]),
                     _ap(vecs[:], 2, [[0, 2], [1, 2]]))
        # negations: 10..12 = -(ckN,ckA,ckC) ; 13 = -skA ; 14,15 = -(7,8)
        V.tensor_scalar(_ap(vecs[:], 10, [[1, 3]]),
                        _ap(vecs[:], 0, [[2, 3]]), -1.0, None, op0=OP.mult)
        V.tensor_scalar(vc(13), vc(3), -1.0, None, op0=OP.mult)
        V.tensor_scalar(_ap(vecs[:], 14, [[1, 2]]),
                        _ap(vecs[:], 7, [[1, 2]]), -1.0, None, op0=OP.mult)
        # 16,17 = RN * (ckN, skN) ; 18,19 = RCA, RC
        V.tensor_mul(_ap(vecs[:], 16, [[1, 2]]),
                     _ap(Rv[:], 0, [[0, 2]]), _ap(vecs[:], 0, [[1, 2]]))
        V.tensor_copy(_ap(vecs[:], 18, [[1, 2]]), Rv[0:1, 1:3])
        G.partition_broadcast(Vb[:], vecs[:])


        S = {}
        for i, nm in enumerate(("ckN", "skN", "ckA", "skA", "ckC", "skC",
                                "ckNckA", "ckNskA", "skNckA", "skNskA",
                                "nckN", "nckA", "nckC", "nskA",
                                "nckNskA", "nskNckA", "RNckN", "RNskN",
                                "RCA", "RC")):
            S[nm] = Vb[:, i:i + 1]

        # ============ Phase D: C1 = B_N @ B_CA (fp16) ============
        cN, sN = ca[:, 0:NR], sa[:, 0:NR]
        cA, sA = ca[:, NR:2 * NR], sa[:, NR:2 * NR]
        cC, sC = ca[:, 2 * NR:3 * NR], sa[:, 2 * NR:3 * NR]

        def blk(t, e, lo=0, hi=NR):
            return t[:, e * NR + lo:e * NR + hi]

        pp1 = PP[:, 0 * NR:1 * NR]
        pp2 = PP[:, 1 * NR:2 * NR]
        pp3 = PP[:, 2 * NR:3 * NR]
        pp4 = PP[:, 3 * NR:4 * NR]
        V.tensor_mul(pp1, cN, cA)
        V.tensor_mul(pp2, sN, sA)
        G.tensor_mul(pp3, cN, sA)
        G.tensor_mul(pp4, sN, cA)
        c1 = C1[:]
        TS(blk(c1, 0), cA, S["skNskA"], S["ckNckA"],
           op0=OP.mult, op1=OP.add)                       # C1_00
        TS(blk(c1, 1), cA, S["nskNckA"], S["ckNskA"],
           op0=OP.mult, op1=OP.add)                       # C1_01
        V.tensor_scalar_mul(blk(c1, 2), sA, S["skN"])     # C1_02
        x = blk(c1, 3)                                    # C1_10
        A.mul(x, cN, S["skNckA"])
        STT(x, pp1, S["nckNskA"], x, op0=OP.mult, op1=OP.add)
        STT(x, pp2, S["skA"], x, op0=OP.mult, op1=OP.add)
        x = blk(c1, 4)                                    # C1_11
        A.mul(x, cN, S["skNskA"])
        STT(x, pp1, S["ckNckA"], x, op0=OP.mult, op1=OP.add)
        STT(x, pp2, S["nckA"], x, op0=OP.mult, op1=OP.add)
        STT(blk(c1, 5), pp3, S["nckN"], pp4,
            op0=OP.mult, op1=OP.subtract)                 # C1_12
        x = blk(c1, 6)                                    # C1_20
        A.mul(x, sN, S["skNckA"])
        STT(x, pp4, S["nckNskA"], x, op0=OP.mult, op1=OP.add)
        STT(x, pp3, S["nskA"], x, op0=OP.mult, op1=OP.add)
        x = blk(c1, 7)                                    # C1_21
        A.mul(x, sN, S["skNskA"])
        STT(x, pp4, S["ckNckA"], x, op0=OP.mult, op1=OP.add)
        STT(x, pp3, S["ckA"], x, op0=OP.mult, op1=OP.add)
        STT(blk(c1, 8), pp2, S["nckN"], pp1,
            op0=OP.mult, op1=OP.add)                      # C1_22

        # residue-0 of q=0 rows: C1 := B_CA(0) (alpha_CA(0)=0)
        r0s = slice(0, BPC)
        o1 = CON16[r0s, C16_ONESR:C16_ONESR + 1]
        V.tensor_scalar_mul(c1[r0s, 0 * NR:0 * NR + 1], o1, S["ckA"][r0s])
        V.tensor_scalar_mul(c1[r0s, 1 * NR:1 * NR + 1], o1, S["skA"][r0s])
        V.memset(c1[r0s, 2 * NR:2 * NR + 1], 0.0)
        V.tensor_scalar_mul(c1[r0s, 3 * NR:3 * NR + 1], o1, S["skA"][r0s])
        V.tensor_scalar_mul(c1[r0s, 4 * NR:4 * NR + 1], o1, S["nckA"][r0s])
        V.memset(c1[r0s, 5 * NR:5 * NR + 1], 0.0)
        V.memset(c1[r0s, 6 * NR:6 * NR + 1], 0.0)
        V.memset(c1[r0s, 7 * NR:7 * NR + 1], 0.0)
        V.memset(c1[r0s, 8 * NR:8 * NR + 1], -1.0)

        # ============ Phase E: Rres = C1 @ B_C -> RA (fp16), vm ========
        ra = RA[:]
        # i-merged: c1 col-planes {o, o+3, o+6} at stride 3*NR
        def c1p(o):
            return _ap(c1, o * NR, [[3 * NR, 3], [1, NR]])

        def rap(o):
            return _ap(ra, o * NR, [[3 * NR, 3], [1, NR]])

        cCb = _ap(ca[:], 2 * NR, [[0, 3], [1, NR]])
        sCb = _ap(sa[:], 2 * NR, [[0, 3], [1, NR]])
        q1m = QQ[:, 0:3 * NR]
        q2m = QQ[:, 3 * NR:6 * NR]
        qsm = T9w3[:, 0:3 * NR]
        V.tensor_mul(q1m, c1p(1), cCb)
        G.tensor_mul(q2m, c1p(2), sCb)
        V.tensor_add(qsm, q1m, q2m)
        A.mul(rap(0), c1p(0), S["ckC"])
        STT(rap(0), qsm, S["skC"], rap(0), op0=OP.mult, op1=OP.add)
        A.mul(rap(1), c1p(0), S["skC"])
        STT(rap(1), qsm, S["nckC"], rap(1), op0=OP.mult, op1=OP.add)
        t1m = T9w1[:, 0:3 * NR]
        t2m = T9w2[:, 0:3 * NR]
        V.tensor_mul(t1m, c1p(1), sCb)
        G.tensor_mul(t2m, c1p(2), cCb)
        V.tensor_sub(rap(2), t1m, t2m)

        # v-vectors (fp16): vm plane p = 3*vec + coord
        vmv = vm[:]
        V.tensor_scalar_mul(blk(vmv, 0), onesr, S["RNckN"])
        A.mul(blk(vmv, 1), cN, S["RNskN"])
        A.mul(blk(vmv, 2), sN, S["RNskN"])
        A.mul(_ap(vmv, 3 * NR, [[1, 3 * NR]]), c1p(0), S["RCA"])
        A.mul(_ap(vmv, 6 * NR, [[1, 3 * NR]]), rap(0), S["RC"])
        # US g0 block = vm m0 (feeds the U expansion's first half)
        A.copy(_ap(US[:], 0, [[128, 9], [1, 32]]),
               _ap(vmv, 0, [[NR, 9], [1, 32]]))

        # ============ setup: mask (selectors come from const DMA) ========
        V.tensor_copy(Lf[:], Lsb[:])
        nc.tensor.matmul(PSL[:, 0:1], selbTv, Lf[:, 0:1],
                         start=True, stop=True)
        for q in range(QN):
            TS(thr[q * BPC:(q + 1) * BPC, 0:1],
               PSL[q * BPC:(q + 1) * BPC, 0:1],
               3.0, float(q * W), op0=OP.mult, op1=OP.subtract)
        TS(maskp[:], jplane, thr[:, 0:1], None, op0=OP.is_lt)

        # ============ Phase F: scan machinery (fp16) ============
        # generic fused 3-term matmul group, col-split DVE/Pool
        def fused(Lf_, Rf_, Of_, Tf_, n, split=0.85, ta=None, tb=None,
                  tcx=None, eng=None):
            ta = T9a if ta is None else ta
            tb = T9b if tb is None else tb
            tcx = (T9c if ta is T9a else T9cf) if tcx is None else tcx
            if eng is not None:
                segs = [(eng, 0, n)]
            else:
                cut = min(n, max(0, int(n * split)))
                segs = []
                if cut > 0:
                    segs.append((V, 0, cut))
                if cut < n:
                    segs.append((G, cut, n))
            for E, c0, c1_ in segs:
                E.tensor_mul(Tf_(ta, c0, c1_), Lf_(0, c0, c1_),
                             Rf_(0, c0, c1_))
                E.tensor_mul(Tf_(tb, c0, c1_), Lf_(1, c0, c1_),
                             Rf_(1, c0, c1_))
                E.tensor_mul(Tf_(tcx, c0, c1_), Lf_(2, c0, c1_),
                             Rf_(2, c0, c1_))
                E.tensor_add(Of_(c0, c1_), Tf_(ta, c0, c1_),
                             Tf_(tb, c0, c1_))
                E.tensor_add(Of_(c0, c1_), Of_(c0, c1_), Tf_(tcx, c0, c1_))

        # --- pair: P2[b, j] = RA[blk 2b, j] @ RA[blk 2b+1, j]
        for b in range(2):
            base = 64 * b

            def PL(k, c0, c1_, base=base):
                return _ap(ra, k * NR + base + c0,
                           [[3 * NR, 3], [0, 3], [1, c1_ - c0]])

            def PR(k, c0, c1_, base=base):
                return _ap(ra, 3 * k * NR + base + NB + c0,
                           [[0, 3], [NR, 3], [1, c1_ - c0]])

            def PO(c0, c1_, base=32 * b):
                return _ap(P2[:], base + c0,
                           [[192, 3], [64, 3], [1, c1_ - c0]])

            def PT(t, c0, c1_, base=288 * b):
                return _ap(t[:], base + c0, [[96, 3], [32, 3], [1, c1_ - c0]])

            fused(PL, PR, PO, PT, NB, eng=V)

        # --- Wodd emitters
        def emit_wodd(b, eng):
            dst, dstride = ((US, 128), (WS, 64))[b]
            base = 64 * b

            def WL(k, c0, c1_, base=base):
                return _ap(ra, k * NR + base + c0,
                           [[3 * NR, 3], [0, 3], [1, c1_ - c0]])

            def WR(k, c0, c1_, base=base):
                return _ap(vmv, k * NR + base + NB + c0,
                           [[0, 3], [3 * NR, 3], [1, c1_ - c0]])

            def WO(c0, c1_, dst=dst, ds=dstride):
                return _ap(dst[:], NB + c0,
                           [[ds, 3], [3 * ds, 3], [1, c1_ - c0]])

            def WT(t, c0, c1_, base=576 + 288 * b):
                return _ap(t[:], base + c0,
                           [[32, 3], [96, 3], [1, c1_ - c0]])

            if eng is None:
                fused(WL, WR, WO, WT, NB, split=0.3)
            else:
                fused(WL, WR, WO, WT, NB, eng=eng)

        # --- quad: P4[j] = P2[b0, j] @ P2[b1, j]
        def QL(k, c0, c1_):
            return _ap(P2[:], k * 64 + c0, [[192, 3], [0, 3], [1, c1_ - c0]])

        def QR(k, c0, c1_):
            return _ap(P2[:], 3 * k * 64 + NB + c0,
                       [[0, 3], [64, 3], [1, c1_ - c0]])

        def QO(c0, c1_):
            return _ap(P4A[:], c0, [[96, 3], [32, 3], [1, c1_ - c0]])

        def QT(t, c0, c1_):
            return _ap(t[:], c0, [[96, 3], [32, 3], [1, c1_ - c0]])

        fused(QL, QR, QO, QT, NB, split=0.84)

        # --- W2 merged: out US cols 64:128 = P2even @ WS
        def emit_w2():
            A.copy(_ap(P2e2[:], 0, [[64, 9], [32, 2], [1, 32]]),
                   _ap(P2[:], 0, [[64, 9], [0, 2], [1, 32]]))
            A.copy(_ap(WS[:], 0, [[64, 9], [1, 32]]),
                   _ap(vmv, 64, [[NR, 9], [1, 32]]))

            def W2L(k, c0, c1_):
                return _ap(P2e2[:], k * 64 + c0,
                           [[192, 3], [0, 3], [1, c1_ - c0]])

            def W2R(k, c0, c1_):
                return _ap(WS[:], k * 64 + c0,
                           [[0, 3], [192, 3], [1, c1_ - c0]])

            def W2O(c0, c1_):
                return _ap(US[:], 64 + c0,
                           [[128, 3], [384, 3], [1, c1_ - c0]])

            def W2T(t, c0, c1_):
                return _ap(t[:], c0, [[64, 3], [192, 3], [1, c1_ - c0]])

            fused(W2L, W2R, W2O, W2T, 2 * NB, split=0.9,
                  ta=T9w1, tb=T9w2, tcx=T9w3)

        # --- Hillis-Steele over 32 quads, expansions interleaved
        def hs_step(srcb, dstb, s):
            n = NB - s
            sv = srcb.rearrange("p (e j) -> p e j", e=9)
            dv = dstb.rearrange("p (e j) -> p e j", e=9)
            V.tensor_copy(dv[:, :, 0:s], sv[:, :, 0:s])

            def HL(k, c0, c1_):
                return _ap(srcb, k * NB + c0,
                           [[96, 3], [0, 3], [1, c1_ - c0]])

            def HR(k, c0, c1_):
                return _ap(srcb, 3 * k * NB + s + c0,
                           [[0, 3], [32, 3], [1, c1_ - c0]])

            def HO(c0, c1_):
                return _ap(dstb, s + c0, [[96, 3], [32, 3], [1, c1_ - c0]])

            def HT(t, c0, c1_):
                return _ap(t[:], c0, [[96, 3], [32, 3], [1, c1_ - c0]])

            fused(HL, HR, HO, HT, n, split=0.8)

        def emit_u(u0, u1):
            def UL(k, c0, c1_):
                return _ap(L4v, k * 128 + u0 + c0,
                           [[384, 3], [0, 3], [1, c1_ - c0]])

            def UR(k, c0, c1_):
                return _ap(US[:], k * 128 + u0 + c0,
                           [[0, 3], [384, 3], [1, c1_ - c0]])

            def UO(c0, c1_):
                return _ap(Uloc[:], u0 + c0,
                           [[384, 3], [128, 3], [1, c1_ - c0]])

            def UT(t, c0, c1_):
                return _ap(t[:], u0 + c0,
                           [[384, 3], [128, 3], [1, c1_ - c0]])

            fused(UL, UR, UO, UT, u1 - u0, split=0.8)

        def emit_ub(u0, u1):
            def UL(k, c0, c1_):
                return _ap(L4v, k * 128 + u0 + c0,
                           [[384, 3], [0, 3], [1, c1_ - c0]])

            def UR(k, c0, c1_):
                return _ap(US[:], k * 128 + u0 + c0,
                           [[0, 3], [384, 3], [1, c1_ - c0]])

            def UO(c0, c1_):
                return _ap(Uloc[:], u0 + c0,
                           [[384, 3], [128, 3], [1, c1_ - c0]])

            def UT(t, c0, c1_):
                return _ap(t[:], u0 + c0,
                           [[384, 3], [128, 3], [1, c1_ - c0]])

            fused(UL, UR, UO, UT, u1 - u0, split=0.9)

        bufs = [P4A, P4B]
        emit_wodd(0, None)
        hs_step(bufs[0][:], bufs[1][:], 1)
        hs_step(bufs[1][:], bufs[0][:], 2)
        hs_step(bufs[0][:], bufs[1][:], 4)
        hs_step(bufs[1][:], bufs[0][:], 8)
        # L4 prefix cols 0:16 are final after HS4 -> copy during HS5 (DVE)
        V.tensor_copy(_ap(L4v, 1, [[128, 9], [32, 4], [1, 16]]),
                      _ap(bufs[0][:], 0, [[NB, 9], [0, 4], [1, 16]]))
        hs_step(bufs[0][:], bufs[1][:], 16)
        emit_wodd(1, V)
        Rscan = bufs[1][:]    # P4B
        V.tensor_copy(_ap(L4v, 17, [[128, 9], [32, 4], [1, NB - 17]]),
                      _ap(Rscan, 16, [[NB, 9], [0, 4], [1, NB - 17]]))
        emit_u(0, 2 * NB)       # g0/g1 half (needs only Wodd b0 + vm)
        emit_w2()

        # --- rotation fixup: E gathered to ALL rows, F chain, slice Fbc ---
        for q in range(QN):
            nc.tensor.matmul(
                PSg[:, q * 9:(q + 1) * 9],
                CON16[:, C16_SELBQ + q * 128:C16_SELBQ + (q + 1) * 128],
                _ap(Rscan, NB - 1, [[NB, 9]]), start=True, stop=True)
        V.tensor_copy(Estack[:, 0:36], PSg[:, 0:36])
        V.tensor_copy(Fstack[:, 0:9], Estack[:, 0:9])
        fs = Fstack[:]
        es = Estack[:]

        def ap2(base_ap, off, dims):
            return _ap(base_ap, off, dims)

        for q in (1, 2):
            FL = lambda k: ap2(fs, (q - 1) * 9 + k, [[3, 3], [0, 3]])
            ER = lambda k: ap2(es, q * 9 + 3 * k, [[0, 3], [1, 3]])
            MT = lambda t: ap2(t[:], 0, [[3, 3], [1, 3]])
            FO = ap2(fs, q * 9, [[3, 3], [1, 3]])
            V.tensor_mul(MT(mt0), FL(0), ER(0))
            V.tensor_mul(MT(mt1), FL(1), ER(1))
            V.tensor_add(MT(mt0), MT(mt0), MT(mt1))
            V.tensor_mul(MT(mt1), FL(2), ER(2))
            V.tensor_add(FO, MT(mt0), MT(mt1))
        # Fbc: rows 0:32 identity; quarter q rows take F_q slice
        V.memset(Fbc[0:BPC, 0:9], 0.0)
        V.memset(bass.AP(Fbc[:].tensor, Fbc[:].offset,
                         [[Fbc[:].ap[0][0], BPC], [4, 3]]), 1.0)
        for q in (1, 2, 3):
            V.tensor_copy(Fbc[q * BPC:(q + 1) * BPC, 0:9],
                          Fstack[q * BPC:(q + 1) * BPC,
                                 (q - 1) * 9:q * 9])

        # --- U merged: Uloc[c][j*128 + g*32 + r] = L4 @ US
        emit_ub(2 * NB, 4 * NB)  # g2/g3 half (needs W2), DVE only

        # seeds: r=0 of each g-block: identity prefix -> u = US col g*32
        V.tensor_copy(_ap(Uloc[:], 0, [[384, 3], [128, 3], [32, 4]]),
                      _ap(US[:], 0, [[128, 3], [384, 3], [32, 4]]))
        # atom 0 of whole chain (q=0 rows) at origin
        V.memset(bass.AP(Uloc[:].tensor, Uloc[:].offset,
                         [[Uloc[:].ap[0][0], BPC], [384, 3]]), 0.0)

        # ============ Phase G: tail ============
        # F-apply fused with blocked->ordered reorder:
        #   Uord_c[a] = F_c0*Ux[b(a)] + F_c1*Uy[b(a)] + F_c2*Uz[b(a)]
        # (walk order (r,g,j): out stride-1-ish ordered, ins blocked)
        def ordw(t, c):
            return _ap(t[:], c * W, [[12, 32], [3, 4], [1, 3]])

        def blkw(t, c):
            return _ap(t[:], c * W, [[1, 32], [32, 4], [128, 3]])

        # --- quarter increments BEFORE the scans (from local sums):
        #   Tloc_c = sum over row of Uloc plane c (full quarter; a partial
        #   quarter's sum only feeds fully-masked later quarters)
        #   S = Fbc @ Tloc (world frame), Pinc = SELcum-gather of S.
        for c in range(3):
            V.tensor_scalar(T9a[:, c * W:(c + 1) * W],
                            Uloc[:, c * W:(c + 1) * W], 1.0, 0.0,
                            op0=OP.mult, op1=OP.add,
                            accum_out=Tloc[:, c:c + 1])
        V.tensor_mul(S9[:, 0:9], Fbc[:, 0:9],
                     _ap(Tloc[:], 0, [[0, 3], [1, 3]]))
        V.tensor_add(S3[:, 0:3], _ap(S9[:], 0, [[3, 3]]),
                     _ap(S9[:], 1, [[3, 3]]))
        V.tensor_add(S3[:, 0:3], S3[:, 0:3], _ap(S9[:], 2, [[3, 3]]))
        nc.tensor.matmul(PSc[:, 0:3], SELcum, S3[:, 0:3],
                         start=True, stop=True)
        V.tensor_copy(FtPbS[:, 0:3], PSc[:, 0:3])

        for c in range(3):
            x = T9x[:, c * W:(c + 1) * W]    # fp16 scratch (blocked)
            y = T9y[:, c * W:(c + 1) * W]
            z = T9z[:, c * W:(c + 1) * W]
            TS(x, Uloc[:, 0:W], Fbc[:, 3 * c + 0:3 * c + 1], None,
               op0=OP.mult)
            TS(y, Uloc[:, W:2 * W], Fbc[:, 3 * c + 1:3 * c + 2], None,
               op0=OP.mult)
            A.mul(z, Uloc[:, 2 * W:3 * W], Fbc[:, 3 * c + 2:3 * c + 3])
            V.tensor_add(x, x, y)
            RS = 20
            V.tensor_add(_ap(Uord[:], c * W, [[12, RS], [3, 4], [1, 3]]),
                         _ap(x, 0, [[1, RS], [32, 4], [128, 3]]),
                         _ap(z, 0, [[1, RS], [32, 4], [128, 3]]))
            G.tensor_add(
                _ap(Uord[:], c * W + 12 * RS, [[12, 32 - RS], [3, 4], [1, 3]]),
                _ap(x, RS, [[1, 32 - RS], [32, 4], [128, 3]]),
                _ap(z, RS, [[1, 32 - RS], [32, 4], [128, 3]]))

        # masked scans seeded with Pinc -> final output directly (fp16)
        for c in range(3):
            V.tensor_tensor_scan(
                Pall[:, c * W:(c + 1) * W], Uord[:, c * W:(c + 1) * W],
                maskp[:], FtPbS[:, c:c + 1], op0=OP.add, op1=OP.mult)
            (nc.sync if c != 1 else nc.gpsimd).dma_start(
                out[:, c * W:(c + 1) * W], Pall[:, c * W:(c + 1) * W])


def _prep_alpha(input):
    # alphaN[r]=psi[r-1], alphaCA[r]=omega[r-1] (0 at r=0), alphaC[r]=phi[r];
    # then mod-4 block permutation within each 128-residue quarter.
    phi, psi, om = input[:, 0], input[:, 1], input[:, 2]
    z1 = np.zeros((input.shape[0], 1), np.float32)
    aN = np.concatenate([z1, psi[:, :-1]], axis=1)
    aCA = np.concatenate([z1, om[:, :-1]], axis=1)
    alpha = np.stack([aN, aCA, phi], axis=1)          # [B, 3, 512]
    alpha = alpha.reshape(-1, 3, QN, NR)
    perm = np.arange(NR).reshape(NB, 4).T.reshape(-1)  # sigma^-1: col->r
    alpha = alpha[..., perm]                           # blocked columns
    return alpha.transpose(0, 2, 1, 3)                 # [B, QN, 3, NR]


def _shard_alpha(alpha, i):
    sl = slice(i * BPC, (i + 1) * BPC)
    return np.ascontiguousarray(
        alpha[sl].transpose(1, 0, 2, 3).reshape(QN * BPC, 3 * NR))


def _build_consts():
    p = np.arange(128)
    i = np.arange(128)
    c16 = np.zeros((128, C16_N), np.float16)
    for q in range(QN):
        c16[:, C16_SELBQ + q * 128:C16_SELBQ + (q + 1) * 128] = (
            (i[None, :] % 32) == (p[:, None] - 32 * q))
    c16[:, C16_ONESR:C16_ONESR + NR] = 1.0
    c32 = np.zeros((128, C32_N), np.float32)
    c32[:, C32_SELCUM:C32_SELCUM + 128] = (
        (p[:, None] % 32 == i[None, :] % 32)
        & (p[:, None] // 32 < i[None, :] // 32))
    c32[0:BPC, C32_SELBT:C32_SELBT + 128] = (
        i[None, :] % 32 == np.arange(BPC)[:, None])
    c32[:, C32_JPLANE:C32_JPLANE + W] = np.arange(W)[None, :]
    return c16, c32


def _get_nc():
    if "nc" not in _CACHE:
        _CACHE["nc"] = _build_graph()
    return _CACHE["nc"]


def kernel(input, param, angles_length, trace=False):
    input = np.ascontiguousarray(input, dtype=np.float32)
    param = np.ascontiguousarray(param, dtype=np.float32)
    angles_length = np.ascontiguousarray(angles_length, dtype=np.int32)
    nc = _get_nc()
    alpha = _prep_alpha(input)
    if "consts" not in _CACHE:
        _CACHE["consts"] = _build_consts()
    c16, c32 = _CACHE["consts"]
    in_maps = []
    for i in range(NCORES):
        sl = slice(i * BPC, (i + 1) * BPC)
        in_maps.append({
            "input": _shard_alpha(alpha, i),
            "param": param,
            "angles_length": angles_length[sl],
            "c16": c16,
            "c32": c32,
        })
    res = run_bass_kernel_spmd(nc, in_maps, core_ids=list(range(NCORES)),
                               trace=trace)
    kernel._last_res = res
    outs = []
    for i in range(NCORES):
        r = res.results[i]["out"]          # [(q,b), (c,j)]
        r = r.reshape(QN, BPC, 3, W)
        r = np.transpose(r, (1, 0, 3, 2)).reshape(BPC, 3 * QN * W)
        outs.append(r)
    full = np.concatenate(outs, axis=0).astype(np.float32)
    if trace:
        kernel._last_exec_ns = res.exec_time_ns
    return full


kernel._last_exec_ns = None



# revision 32
# speedup vs baseline: 1.0057x; 1.0057x over previous
"""Trainium2 Bass kernel for Angles2Backbone (NeRF chain forward).

Full inputs: input [256,3,512] f32, param [6] f32, angles_length [256] i32.
Output: [256, 4608] f32  (coords of 1536 backbone atoms x 3, masked).

Sharding: pure data parallel over batch - 32 proteins per core x 8 cores.

Per-core algorithm (v3: fp16 scan machinery, mod-4 blocked residue layout):
  - 128 partitions = (quarter q)*32 + protein b; each row owns 128 residues.
  - Residue r of a quarter lives at column sigma(r) = 32*(r%4) + r//4
    (host-side permutation), so every pair/quad/expansion op reads and
    writes stride-1 column blocks -> DVE 2x fp16 mode throughout.
  - Trig via sin LUT at a/8, a/4 + double-angle chains (fp16).
  - Pre-pass builds per-residue rotation Rres (fp16) from scalar-folded
    bilinear terms; v-vectors (per-atom translations) in fp16.
  - pair (mod-4 blocks) -> P2, quad -> P4, Hillis-Steele over 32 quads.
  - Vector expansions Wodd/W2 written straight into a fused source tensor
    US so the superquad expansion is 5 big ops against a replicated,
    shifted prefix tensor L4.
  - Positions: blocked u -> reorder copy -> masked tensor_tensor_scan
    seeded with F^T.Pinc (cross-quarter fixup folded into the scan),
    then frame rotation F and store.
  - Cross-partition moves via PE matmuls only (no SBUF-SBUF DMA).
"""

import sys

sys.path.insert(0, "/opt/trn_rl_repo")

import numpy as np
import concourse.bass as bass
import concourse.bacc as bacc
import concourse.mybir as mybir
from concourse import tile
from concourse.bass_utils import run_bass_kernel_spmd

F32 = mybir.dt.float32
F16 = mybir.dt.float16
I32 = mybir.dt.int32
AF = mybir.ActivationFunctionType
OP = mybir.AluOpType

NCORES = 8
BPC = 32          # proteins per core
L = 512           # residues per protein
QN = 4            # chain quarters per protein (partition groups)
W = 384           # atoms per quarter
NR = 128          # residues per quarter
NB = 32           # columns per mod-4 block
PI = float(np.pi)

_CACHE = {}


# const block layouts (per partition)
C16_SELBQ = 0          # [512] selbq one-hots (fp16)
C16_L4 = 512           # [1152] L4 zero-init (fp16)
C16_ONESR = 1664       # [128] ones (fp16)
C16_N = 1792
C32_SELCUM = 0         # [128] cumulative masked selector (f32)
C32_SELBT = 128        # [128] selbT rows 0:32 (f32)
C32_JPLANE = 256       # [384] atom index (f32)
C32_N = 640


def _build_graph():
    nc = bacc.Bacc("TRN2", target_bir_lowering=False, debug=False,
                   num_devices=NCORES)
    inp = nc.dram_tensor("input", [QN * BPC, 3 * NR], F32,
                         kind="ExternalInput").ap()
    par = nc.dram_tensor("param", [6], F32, kind="ExternalInput").ap()
    alen = nc.dram_tensor("angles_length", [BPC], I32,
                          kind="ExternalInput").ap()
    c16 = nc.dram_tensor("c16", [128, C16_N], F16,
                         kind="ExternalInput").ap()
    c32 = nc.dram_tensor("c32", [128, C32_N], F32,
                         kind="ExternalInput").ap()
    out = nc.dram_tensor("out", [QN * BPC, 3 * W], F16,
                         kind="ExternalOutput").ap()
    with tile.TileContext(nc) as tc:
        _emit(nc, tc, inp, par, alen, c16, c32, out)
    nc.compile()
    return nc


def _ap(base_ap, off, dims):
    return bass.AP(base_ap.tensor, base_ap.offset + off,
                   [list(base_ap.ap[0])] + [list(d) for d in dims])


def _emit(nc, tc, inp, par, alen, c16, c32, out):
    import contextlib
    ctx = contextlib.ExitStack()
    with ctx:
        main = ctx.enter_context(tc.tile_pool(name="main", bufs=1))
        psum = ctx.enter_context(tc.tile_pool(name="psum", bufs=1,
                                              space="PSUM"))

        # ---------------- tiles ----------------
        alpha = main.tile([128, 3 * NR], F32, tag="alpha")
        ca = main.tile([128, 3 * NR], F16, tag="ca")
        sa = main.tile([128, 3 * NR], F16, tag="sa")
        # trig scratch (magic-number range reduction)
        tsq = main.tile([128, 3 * NR], F32, tag="tsq")   # squares scratch
        ts2 = main.tile([128, 3 * NR], F32, tag="ts2")   # scratch
        tmagic = main.tile([128, 3 * NR], F32, tag="tmagic")
        thalf = main.tile([128, 3 * NR], F32, tag="thalf")

        PP = main.tile([128, 4 * NR], F16, tag="PP")
        C1 = main.tile([128, 9 * NR], F16, tag="C1")
        QQ = main.tile([128, 6 * NR], F16, tag="QQ")
        RA = main.tile([128, 9 * NR], F16, tag="RA")     # Rres fp16
        vm = main.tile([128, 9 * NR], F16, tag="vm")
        P2 = main.tile([128, 9 * 2 * NB], F16, tag="P2")
        P2e2 = main.tile([128, 9 * 2 * NB], F16, tag="P2e2")
        P4A = main.tile([128, 9 * NB], F16, tag="P4A")
        P4B = main.tile([128, 9 * NB], F16, tag="P4B")
        US = main.tile([128, 9 * 4 * NB], F16, tag="US")
        WS = main.tile([128, 9 * 2 * NB], F16, tag="WS")
        T9a = main.tile([128, 9 * 4 * NB], F16, tag="T9a")
        T9b = main.tile([128, 9 * 4 * NB], F16, tag="T9b")
        T9c = main.tile([128, 9 * 4 * NB], F16, tag="T9c")
        T9w1 = main.tile([128, 9 * 2 * NB], F16, tag="T9w1")
        T9w2 = main.tile([128, 9 * 2 * NB], F16, tag="T9w2")
        T9w3 = main.tile([128, 9 * 2 * NB], F16, tag="T9w3")
        T9x = main.tile([128, 3 * W], F16, tag="T9x")
        T9y = main.tile([128, 3 * W], F16, tag="T9y")
        T9z = main.tile([128, 3 * W], F16, tag="T9z")
        T9af = main.tile([128, 18 * NB], F32, tag="T9af")
        T9bf = main.tile([128, 18 * NB], F32, tag="T9bf")
        T9cf = main.tile([128, 18 * NB], F32, tag="T9cf")
        Uloc = main.tile([128, 3 * W], F16, tag="Uloc")  # blocked u
        Uord = main.tile([128, 3 * W], F16, tag="Uord")  # ordered u
        Pall = main.tile([128, 3 * W], F16, tag="Pall")  # scanned+seeded out

        maskp = main.tile([128, W], F16, tag="maskp")
        thr = main.tile([128, 1], F32, tag="thr")
        Lsb = main.tile([BPC, 1], I32, tag="Lsb")
        Lf = main.tile([BPC, 1], F32, tag="Lf")
        Psb = main.tile([1, 6], F32, tag="Psb")
        kv = main.tile([1, 3], F32, tag="kv")
        Rv = main.tile([1, 3], F32, tag="Rv")
        NSC = 24
        vecs = main.tile([1, NSC], F32, tag="vecs")
        Vb = main.tile([128, NSC], F32, tag="Vb")
        zb1 = main.tile([1, 1], F32, tag="zb1")
        zb128 = main.tile([128, 1], F32, tag="zb128")
        warm = main.tile([1, 1], F32, tag="warm")

        # DMA'd constants (selbq one-hots, L4 zero init, ones, SELcum,
        # selbT, jplane)
        CON16 = main.tile([128, C16_N], F16, tag="CON16")
        CON32 = main.tile([128, C32_N], F32, tag="CON32")
        L4v = CON16[:, C16_L4:C16_L4 + 9 * 4 * NB]
        onesr = CON16[:, C16_ONESR:C16_ONESR + NR]
        SELcum = CON32[:, C32_SELCUM:C32_SELCUM + 128]
        selbTv = CON32[0:BPC, C32_SELBT:C32_SELBT + 128]
        jplane = CON32[:, C32_JPLANE:C32_JPLANE + W]

        # cross-quarter fixup (redundantly on all 128 rows, f32)
        Estack = main.tile([128, 36], F32, tag="Estack")
        Fstack = main.tile([128, 27], F32, tag="Fstack")
        Fbc = main.tile([128, 9], F32, tag="Fbc")
        mt0 = main.tile([128, 9], F32, tag="mt0")
        mt1 = main.tile([128, 9], F32, tag="mt1")
        # tail: local sums -> world quarter increments
        Tloc = main.tile([128, 3], F32, tag="Tloc")
        S9 = main.tile([128, 9], F32, tag="S9")
        S3 = main.tile([128, 3], F32, tag="S3")
        FtPbS = main.tile([128, 3], F32, tag="FtPbS")

        PSg = psum.tile([128, 36], F32, tag="PSg")
        PSc = psum.tile([128, 3], F32, tag="PSc")
        PSL = psum.tile([128, 1], F32, tag="PSL")

        V = nc.vector
        G = nc.gpsimd
        A = nc.scalar
        STT = nc.vector.scalar_tensor_tensor
        TS = nc.vector.tensor_scalar
        GTS = nc.gpsimd.tensor_scalar

        # ============ Phase A: DMAs + ACT warmup + setup ============
        nc.sync.dma_start(alpha[:], inp[:])
        nc.sync.dma_start(Psb[:], par[:])
        nc.sync.dma_start(Lsb[:], alen[:])
        nc.gpsimd.dma_start(CON16[:], c16[:])
        nc.gpsimd.dma_start(CON32[:], c32[:])
        V.memset(zb1[:], 0.0)
        V.memset(zb128[:], 0.0)
        # trigger the Sin table load immediately (Copy set loads after sins)
        A.activation(warm[:], zb1[:], AF.Sin, bias=zb1[:])

        # ============ Phase B: trig (2^23 magic range reduction + Sin) ====
        # ar = alpha - 2pi*round(alpha/2pi) in [-pi, pi]; sa = sin(ar);
        # ca = 1 - 2*sin(ar/2)^2. 1.5*2^23 keeps the rounding add in the
        # [2^23, 2^24) binade (ulp=1) for negative args too.
        # Per type block (N, CA, C) to pipeline the serial chain. Emitted
        # BEFORE the param-scalar block: DVE dispatch is in-order and the
        # param copies wait on the (later) param DMA sem.
        MAGIC = float(3 * 2 ** 22)

        def trig_dve(t):
            bs = slice(t * NR, (t + 1) * NR)
            V.tensor_scalar(tmagic[:, bs], alpha[:, bs], 1.0 / (2 * PI),
                            MAGIC, op0=OP.mult, op1=OP.add)
            V.tensor_scalar(tmagic[:, bs], tmagic[:, bs], MAGIC, None,
                            op0=OP.subtract)
            STT(ts2[:, bs], tmagic[:, bs], -2 * PI, alpha[:, bs],
                op0=OP.mult, op1=OP.add)

        def trig_sin(t):
            bs = slice(t * NR, (t + 1) * NR)
            A.activation(sa[:, bs], ts2[:, bs], AF.Sin, bias=zb128[:])
            A.activation(thalf[:, bs], ts2[:, bs], AF.Sin, bias=zb128[:],
                         scale=0.5)

        def trig_cos(t):
            bs = slice(t * NR, (t + 1) * NR)
            G.tensor_mul(tsq[:, bs], thalf[:, bs], thalf[:, bs])
            G.tensor_scalar(ca[:, bs], tsq[:, bs], -2.0, 1.0,
                            op0=OP.mult, op1=OP.add)

        trig_dve(0)
        trig_sin(0)
        trig_dve(1)

        # param scalars (wait on the param DMA; emitted between trig blocks)
        for t, idx in enumerate((5, 1, 3)):   # kappa: CA_C_N, C_N_CA, N_CA_C
            V.tensor_copy(kv[0:1, t:t + 1], Psb[0:1, idx:idx + 1])
        for t, idx in enumerate((4, 0, 2)):   # R: R_C_N, R_N_CA, R_CA_C
            V.tensor_copy(Rv[0:1, t:t + 1], Psb[0:1, idx:idx + 1])
        sk3 = main.tile([1, 3], F32, tag="sk3")
        ck3 = main.tile([1, 3], F32, tag="ck3")
        kvr = main.tile([1, 3], F32, tag="kvr")
        A.activation(sk3[:], kv[0:1, 0:3], AF.Sin, bias=zb1[:])
        A.activation(kvr[:], kv[0:1, 0:3], AF.Sin, bias=zb1[:], scale=0.5)
        trig_sin(1)
        trig_cos(0)
        trig_dve(2)
        trig_sin(2)
        trig_cos(1)
        trig_cos(2)
        A.copy(warm[:], zb1[:])     # Copy-set LUT load after the last Sin

        V.tensor_mul(kvr[:], kvr[:], kvr[:])
        V.tensor_scalar(ck3[:], kvr[:], -2.0, 1.0, op0=OP.mult, op1=OP.add)

        # scalar slots in vecs[1, NSC]:
        # 0:ckN 1:skN 2:ckA 3:skA 4:ckC 5:skC
        # 6:ckNckA 7:ckNskA 8:skNckA 9:skNskA
        # 10:nckN 11:nckA 12:nckC 13:nskA 14:nckNskA 15:nskNckA
        # 16:RNckN 17:RNskN 18:RCA 19:RC
        def vc(i):
            return vecs[0:1, i:i + 1]

        # interleave ck/sk into slots 0..5
        V.tensor_copy(_ap(vecs[:], 0, [[2, 3]]), ck3[0:1, 0:3])
        V.tensor_copy(_ap(vecs[:], 1, [[2, 3]]), sk3[0:1, 0:3])
        # outer product (ckN,skN) x (ckA,skA) -> slots 6..9
        V.tensor_mul(_ap(vecs[:], 6, [[2, 2], [1, 2]]),
                     _ap(vecs[:], 0, [[1, 2], [0, 2]]),
                     _ap(vecs[:], 2, [[0, 2], [1, 2]]))
        # negations: 10..12 = -(ckN,ckA,ckC) ; 13 = -skA ; 14,15 = -(7,8)
        V.tensor_scalar(_ap(vecs[:], 10, [[1, 3]]),
                        _ap(vecs[:], 0, [[2, 3]]), -1.0, None, op0=OP.mult)
        V.tensor_scalar(vc(13), vc(3), -1.0, None, op0=OP.mult)
        V.tensor_scalar(_ap(vecs[:], 14, [[1, 2]]),
                        _ap(vecs[:], 7, [[1, 2]]), -1.0, None, op0=OP.mult)
        # 16,17 = RN * (ckN, skN) ; 18,19 = RCA, RC
        V.tensor_mul(_ap(vecs[:], 16, [[1, 2]]),
                     _ap(Rv[:], 0, [[0, 2]]), _ap(vecs[:], 0, [[1, 2]]))
        V.tensor_copy(_ap(vecs[:], 18, [[1, 2]]), Rv[0:1, 1:3])
        G.partition_broadcast(Vb[:], vecs[:])


        S = {}
        for i, nm in enumerate(("ckN", "skN", "ckA", "skA", "ckC", "skC",
                                "ckNckA", "ckNskA", "skNckA", "skNskA",
                                "nckN", "nckA", "nckC", "nskA",
                                "nckNskA", "nskNckA", "RNckN", "RNskN",
                                "RCA", "RC")):
            S[nm] = Vb[:, i:i + 1]

        # ============ Phase D: C1 = B_N @ B_CA (fp16) ============
        cN, sN = ca[:, 0:NR], sa[:, 0:NR]
        cA, sA = ca[:, NR:2 * NR], sa[:, NR:2 * NR]
        cC, sC = ca[:, 2 * NR:3 * NR], sa[:, 2 * NR:3 * NR]

        def blk(t, e, lo=0, hi=NR):
            return t[:, e * NR + lo:e * NR + hi]

        pp1 = PP[:, 0 * NR:1 * NR]
        pp2 = PP[:, 1 * NR:2 * NR]
        pp3 = PP[:, 2 * NR:3 * NR]
        pp4 = PP[:, 3 * NR:4 * NR]
        V.tensor_mul(pp1, cN, cA)
        V.tensor_mul(pp2, sN, sA)
        G.tensor_mul(pp3, cN, sA)
        G.tensor_mul(pp4, sN, cA)
        c1 = C1[:]
        TS(blk(c1, 0), cA, S["skNskA"], S["ckNckA"],
           op0=OP.mult, op1=OP.add)                       # C1_00
        TS(blk(c1, 1), cA, S["nskNckA"], S["ckNskA"],
           op0=OP.mult, op1=OP.add)                       # C1_01
        V.tensor_scalar_mul(blk(c1, 2), sA, S["skN"])     # C1_02
        x = blk(c1, 3)                                    # C1_10
        A.mul(x, cN, S["skNckA"])
        STT(x, pp1, S["nckNskA"], x, op0=OP.mult, op1=OP.add)
        STT(x, pp2, S["skA"], x, op0=OP.mult, op1=OP.add)
        x = blk(c1, 4)                                    # C1_11
        A.mul(x, cN, S["skNskA"])
        STT(x, pp1, S["ckNckA"], x, op0=OP.mult, op1=OP.add)
        STT(x, pp2, S["nckA"], x, op0=OP.mult, op1=OP.add)
        STT(blk(c1, 5), pp3, S["nckN"], pp4,
            op0=OP.mult, op1=OP.subtract)                 # C1_12
        x = blk(c1, 6)                                    # C1_20
        A.mul(x, sN, S["skNckA"])
        STT(x, pp4, S["nckNskA"], x, op0=OP.mult, op1=OP.add)
        STT(x, pp3, S["nskA"], x, op0=OP.mult, op1=OP.add)
        x = blk(c1, 7)                                    # C1_21
        A.mul(x, sN, S["skNskA"])
        STT(x, pp4, S["ckNckA"], x, op0=OP.mult, op1=OP.add)
        STT(x, pp3, S["ckA"], x, op0=OP.mult, op1=OP.add)
        STT(blk(c1, 8), pp2, S["nckN"], pp1,
            op0=OP.mult, op1=OP.add)                      # C1_22

        # residue-0 of q=0 rows: C1 := B_CA(0) (alpha_CA(0)=0)
        r0s = slice(0, BPC)
        o1 = CON16[r0s, C16_ONESR:C16_ONESR + 1]
        V.tensor_scalar_mul(c1[r0s, 0 * NR:0 * NR + 1], o1, S["ckA"][r0s])
        V.tensor_scalar_mul(c1[r0s, 1 * NR:1 * NR + 1], o1, S["skA"][r0s])
        V.memset(c1[r0s, 2 * NR:2 * NR + 1], 0.0)
        V.tensor_scalar_mul(c1[r0s, 3 * NR:3 * NR + 1], o1, S["skA"][r0s])
        V.tensor_scalar_mul(c1[r0s, 4 * NR:4 * NR + 1], o1, S["nckA"][r0s])
        V.memset(c1[r0s, 5 * NR:5 * NR + 1], 0.0)
        V.memset(c1[r0s, 6 * NR:6 * NR + 1], 0.0)
        V.memset(c1[r0s, 7 * NR:7 * NR + 1], 0.0)
        V.memset(c1[r0s, 8 * NR:8 * NR + 1], -1.0)

        # ============ Phase E: Rres = C1 @ B_C -> RA (fp16), vm ========
        ra = RA[:]
        for i in range(3):
            (V if i != 2 else G).tensor_mul(blk(QQ[:], i),
                                            blk(c1, 3 * i + 1), cC)
            (G if i != 2 else V).tensor_mul(blk(QQ[:], 3 + i),
                                            blk(c1, 3 * i + 2), sC)
        for i in range(3):
            q1i = blk(QQ[:], i)
            q2i = blk(QQ[:], 3 + i)
            qs = T9w3[:, i * NR:(i + 1) * NR]
            (V if i != 2 else G).tensor_add(qs, q1i, q2i)
            x = blk(ra, 3 * i + 0)
            A.mul(x, blk(c1, 3 * i + 0), S["ckC"])
            STT(x, qs, S["skC"], x, op0=OP.mult, op1=OP.add)
            x = blk(ra, 3 * i + 1)
            A.mul(x, blk(c1, 3 * i + 0), S["skC"])
            STT(x, qs, S["nckC"], x, op0=OP.mult, op1=OP.add)
            t1 = T9w1[:, i * NR:(i + 1) * NR]
            t2 = T9w2[:, i * NR:(i + 1) * NR]
            V.tensor_mul(t1, blk(c1, 3 * i + 1), sC)
            G.tensor_mul(t2, blk(c1, 3 * i + 2), cC)
            V.tensor_sub(blk(ra, 3 * i + 2), t1, t2)

        # v-vectors (fp16): vm plane p = 3*vec + coord
        vmv = vm[:]
        V.tensor_scalar_mul(blk(vmv, 0), onesr, S["RNckN"])
        A.mul(blk(vmv, 1), cN, S["RNskN"])
        A.mul(blk(vmv, 2), sN, S["RNskN"])
        A.mul(_ap(vmv, 3 * NR, [[1, 3 * NR]]),
              _ap(c1, 0, [[3 * NR, 3], [1, NR]]), S["RCA"])
        A.mul(_ap(vmv, 6 * NR, [[1, 3 * NR]]),
              _ap(ra, 0, [[3 * NR, 3], [1, NR]]), S["RC"])
        # US g0 block = vm m0 (feeds the U expansion's first half)
        A.copy(_ap(US[:], 0, [[128, 9], [1, 32]]),
               _ap(vmv, 0, [[NR, 9], [1, 32]]))

        # ============ setup: mask (selectors come from const DMA) ========
        V.tensor_copy(Lf[:], Lsb[:])
        nc.tensor.matmul(PSL[:, 0:1], selbTv, Lf[:, 0:1],
                         start=True, stop=True)
        for q in range(QN):
            TS(thr[q * BPC:(q + 1) * BPC, 0:1],
               PSL[q * BPC:(q + 1) * BPC, 0:1],
               3.0, float(q * W), op0=OP.mult, op1=OP.subtract)
        TS(maskp[:], jplane, thr[:, 0:1], None, op0=OP.is_lt)

        # ============ Phase F: scan machinery (fp16) ============
        # generic fused 3-term matmul group, col-split DVE/Pool
        def fused(Lf_, Rf_, Of_, Tf_, n, split=0.85, ta=None, tb=None,
                  tcx=None, eng=None):
            ta = T9a if ta is None else ta
            tb = T9b if tb is None else tb
            tcx = (T9c if ta is T9a else T9cf) if tcx is None else tcx
            if eng is not None:
                segs = [(eng, 0, n)]
            else:
                cut = min(n, max(0, int(n * split)))
                segs = []
                if cut > 0:
                    segs.append((V, 0, cut))
                if cut < n:
                    segs.append((G, cut, n))
            for E, c0, c1_ in segs:
                E.tensor_mul(Tf_(ta, c0, c1_), Lf_(0, c0, c1_),
                             Rf_(0, c0, c1_))
                E.tensor_mul(Tf_(tb, c0, c1_), Lf_(1, c0, c1_),
                             Rf_(1, c0, c1_))
                E.tensor_mul(Tf_(tcx, c0, c1_), Lf_(2, c0, c1_),
                             Rf_(2, c0, c1_))
                E.tensor_add(Of_(c0, c1_), Tf_(ta, c0, c1_),
                             Tf_(tb, c0, c1_))
                E.tensor_add(Of_(c0, c1_), Of_(c0, c1_), Tf_(tcx, c0, c1_))

        # --- pair: P2[b, j] = RA[blk 2b, j] @ RA[blk 2b+1, j]
        for b in range(2):
            base = 64 * b

            def PL(k, c0, c1_, base=base):
                return _ap(ra, k * NR + base + c0,
                           [[3 * NR, 3], [0, 3], [1, c1_ - c0]])

            def PR(k, c0, c1_, base=base):
                return _ap(ra, 3 * k * NR + base + NB + c0,
                           [[0, 3], [NR, 3], [1, c1_ - c0]])

            def PO(c0, c1_, base=32 * b):
                return _ap(P2[:], base + c0,
                           [[192, 3], [64, 3], [1, c1_ - c0]])

            def PT(t, c0, c1_, base=288 * b):
                return _ap(t[:], base + c0, [[96, 3], [32, 3], [1, c1_ - c0]])

            fused(PL, PR, PO, PT, NB, eng=V)

        # --- Wodd emitters
        def emit_wodd(b, eng):
            dst, dstride = ((US, 128), (WS, 64))[b]
            base = 64 * b

            def WL(k, c0, c1_, base=base):
                return _ap(ra, k * NR + base + c0,
                           [[3 * NR, 3], [0, 3], [1, c1_ - c0]])

            def WR(k, c0, c1_, base=base):
                return _ap(vmv, k * NR + base + NB + c0,
                           [[0, 3], [3 * NR, 3], [1, c1_ - c0]])

            def WO(c0, c1_, dst=dst, ds=dstride):
                return _ap(dst[:], NB + c0,
                           [[ds, 3], [3 * ds, 3], [1, c1_ - c0]])

            def WT(t, c0, c1_, base=576 + 288 * b):
                return _ap(t[:], base + c0,
                           [[32, 3], [96, 3], [1, c1_ - c0]])

            if eng is None:
                fused(WL, WR, WO, WT, NB, split=0.3)
            else:
                fused(WL, WR, WO, WT, NB, eng=eng)

        # --- quad: P4[j] = P2[b0, j] @ P2[b1, j]
        def QL(k, c0, c1_):
            return _ap(P2[:], k * 64 + c0, [[192, 3], [0, 3], [1, c1_ - c0]])

        def QR(k, c0, c1_):
            return _ap(P2[:], 3 * k * 64 + NB + c0,
                       [[0, 3], [64, 3], [1, c1_ - c0]])

        def QO(c0, c1_):
            return _ap(P4A[:], c0, [[96, 3], [32, 3], [1, c1_ - c0]])

        def QT(t, c0, c1_):
            return _ap(t[:], c0, [[96, 3], [32, 3], [1, c1_ - c0]])

        fused(QL, QR, QO, QT, NB, split=0.84)

        # --- W2 merged: out US cols 64:128 = P2even @ WS
        def emit_w2():
            A.copy(_ap(P2e2[:], 0, [[64, 9], [32, 2], [1, 32]]),
                   _ap(P2[:], 0, [[64, 9], [0, 2], [1, 32]]))
            A.copy(_ap(WS[:], 0, [[64, 9], [1, 32]]),
                   _ap(vmv, 64, [[NR, 9], [1, 32]]))

            def W2L(k, c0, c1_):
                return _ap(P2e2[:], k * 64 + c0,
                           [[192, 3], [0, 3], [1, c1_ - c0]])

            def W2R(k, c0, c1_):
                return _ap(WS[:], k * 64 + c0,
                           [[0, 3], [192, 3], [1, c1_ - c0]])

            def W2O(c0, c1_):
                return _ap(US[:], 64 + c0,
                           [[128, 3], [384, 3], [1, c1_ - c0]])

            def W2T(t, c0, c1_):
                return _ap(t[:], c0, [[64, 3], [192, 3], [1, c1_ - c0]])

            fused(W2L, W2R, W2O, W2T, 2 * NB, split=0.9,
                  ta=T9w1, tb=T9w2, tcx=T9w3)

        # --- Hillis-Steele over 32 quads, expansions interleaved
        def hs_step(srcb, dstb, s):
            n = NB - s
            sv = srcb.rearrange("p (e j) -> p e j", e=9)
            dv = dstb.rearrange("p (e j) -> p e j", e=9)
            V.tensor_copy(dv[:, :, 0:s], sv[:, :, 0:s])

            def HL(k, c0, c1_):
                return _ap(srcb, k * NB + c0,
                           [[96, 3], [0, 3], [1, c1_ - c0]])

            def HR(k, c0, c1_):
                return _ap(srcb, 3 * k * NB + s + c0,
                           [[0, 3], [32, 3], [1, c1_ - c0]])

            def HO(c0, c1_):
                return _ap(dstb, s + c0, [[96, 3], [32, 3], [1, c1_ - c0]])

            def HT(t, c0, c1_):
                return _ap(t[:], c0, [[96, 3], [32, 3], [1, c1_ - c0]])

            fused(HL, HR, HO, HT, n, split=0.8)

        def emit_u(u0, u1):
            def UL(k, c0, c1_):
                return _ap(L4v, k * 128 + u0 + c0,
                           [[384, 3], [0, 3], [1, c1_ - c0]])

            def UR(k, c0, c1_):
                return _ap(US[:], k * 128 + u0 + c0,
                           [[0, 3], [384, 3], [1, c1_ - c0]])

            def UO(c0, c1_):
                return _ap(Uloc[:], u0 + c0,
                           [[384, 3], [128, 3], [1, c1_ - c0]])

            def UT(t, c0, c1_):
                return _ap(t[:], u0 + c0,
                           [[384, 3], [128, 3], [1, c1_ - c0]])

            fused(UL, UR, UO, UT, u1 - u0, split=0.8)

        def emit_ub(u0, u1):
            def UL(k, c0, c1_):
                return _ap(L4v, k * 128 + u0 + c0,
                           [[384, 3], [0, 3], [1, c1_ - c0]])

            def UR(k, c0, c1_):
                return _ap(US[:], k * 128 + u0 + c0,
                           [[0, 3], [384, 3], [1, c1_ - c0]])

            def UO(c0, c1_):
                return _ap(Uloc[:], u0 + c0,
                           [[384, 3], [128, 3], [1, c1_ - c0]])

            def UT(t, c0, c1_):
                return _ap(t[:], u0 + c0,
                           [[384, 3], [128, 3], [1, c1_ - c0]])

            fused(UL, UR, UO, UT, u1 - u0, split=0.9)

        bufs = [P4A, P4B]
        emit_wodd(0, None)
        hs_step(bufs[0][:], bufs[1][:], 1)
        hs_step(bufs[1][:], bufs[0][:], 2)
        hs_step(bufs[0][:], bufs[1][:], 4)
        hs_step(bufs[1][:], bufs[0][:], 8)
        # L4 prefix cols 0:16 are final after HS4 -> copy during HS5 (DVE)
        V.tensor_copy(_ap(L4v, 1, [[128, 9], [32, 4], [1, 16]]),
                      _ap(bufs[0][:], 0, [[NB, 9], [0, 4], [1, 16]]))
        hs_step(bufs[0][:], bufs[1][:], 16)
        emit_wodd(1, V)
        Rscan = bufs[1][:]    # P4B
        V.tensor_copy(_ap(L4v, 17, [[128, 9], [32, 4], [1, NB - 17]]),
                      _ap(Rscan, 16, [[NB, 9], [0, 4], [1, NB - 17]]))
        emit_u(0, 2 * NB)       # g0/g1 half (needs only Wodd b0 + vm)
        emit_w2()

        # --- rotation fixup: E gathered to ALL rows, F chain, slice Fbc ---
        for q in range(QN):
            nc.tensor.matmul(
                PSg[:, q * 9:(q + 1) * 9],
                CON16[:, C16_SELBQ + q * 128:C16_SELBQ + (q + 1) * 128],
                _ap(Rscan, NB - 1, [[NB, 9]]), start=True, stop=True)
        V.tensor_copy(Estack[:, 0:36], PSg[:, 0:36])
        V.tensor_copy(Fstack[:, 0:9], Estack[:, 0:9])
        fs = Fstack[:]
        es = Estack[:]

        def ap2(base_ap, off, dims):
            return _ap(base_ap, off, dims)

        for q in (1, 2):
            FL = lambda k: ap2(fs, (q - 1) * 9 + k, [[3, 3], [0, 3]])
            ER = lambda k: ap2(es, q * 9 + 3 * k, [[0, 3], [1, 3]])
            MT = lambda t: ap2(t[:], 0, [[3, 3], [1, 3]])
            FO = ap2(fs, q * 9, [[3, 3], [1, 3]])
            V.tensor_mul(MT(mt0), FL(0), ER(0))
            V.tensor_mul(MT(mt1), FL(1), ER(1))
            V.tensor_add(MT(mt0), MT(mt0), MT(mt1))
            V.tensor_mul(MT(mt1), FL(2), ER(2))
            V.tensor_add(FO, MT(mt0), MT(mt1))
        # Fbc: rows 0:32 identity; quarter q rows take F_q slice
        V.memset(Fbc[0:BPC, 0:9], 0.0)
        V.memset(bass.AP(Fbc[:].tensor, Fbc[:].offset,
                         [[Fbc[:].ap[0][0], BPC], [4, 3]]), 1.0)
        for q in (1, 2, 3):
            V.tensor_copy(Fbc[q * BPC:(q + 1) * BPC, 0:9],
                          Fstack[q * BPC:(q + 1) * BPC,
                                 (q - 1) * 9:q * 9])

        # --- U merged: Uloc[c][j*128 + g*32 + r] = L4 @ US
        emit_ub(2 * NB, 4 * NB)  # g2/g3 half (needs W2), DVE only

        # seeds: r=0 of each g-block: identity prefix -> u = US col g*32
        V.tensor_copy(_ap(Uloc[:], 0, [[384, 3], [128, 3], [32, 4]]),
                      _ap(US[:], 0, [[128, 3], [384, 3], [32, 4]]))
        # atom 0 of whole chain (q=0 rows) at origin
        V.memset(bass.AP(Uloc[:].tensor, Uloc[:].offset,
                         [[Uloc[:].ap[0][0], BPC], [384, 3]]), 0.0)

        # ============ Phase G: tail ============
        # F-apply fused with blocked->ordered reorder:
        #   Uord_c[a] = F_c0*Ux[b(a)] + F_c1*Uy[b(a)] + F_c2*Uz[b(a)]
        # (walk order (r,g,j): out stride-1-ish ordered, ins blocked)
        def ordw(t, c):
            return _ap(t[:], c * W, [[12, 32], [3, 4], [1, 3]])

        def blkw(t, c):
            return _ap(t[:], c * W, [[1, 32], [32, 4], [128, 3]])

        # --- quarter increments BEFORE the scans (from local sums):
        #   Tloc_c = sum over row of Uloc plane c (full quarter; a partial
        #   quarter's sum only feeds fully-masked later quarters)
        #   S = Fbc @ Tloc (world frame), Pinc = SELcum-gather of S.
        for c in range(3):
            V.tensor_scalar(T9a[:, c * W:(c + 1) * W],
                            Uloc[:, c * W:(c + 1) * W], 1.0, 0.0,
                            op0=OP.mult, op1=OP.add,
                            accum_out=Tloc[:, c:c + 1])
        V.tensor_mul(S9[:, 0:9], Fbc[:, 0:9],
                     _ap(Tloc[:], 0, [[0, 3], [1, 3]]))
        V.tensor_add(S3[:, 0:3], _ap(S9[:], 0, [[3, 3]]),
                     _ap(S9[:], 1, [[3, 3]]))
        V.tensor_add(S3[:, 0:3], S3[:, 0:3], _ap(S9[:], 2, [[3, 3]]))
        nc.tensor.matmul(PSc[:, 0:3], SELcum, S3[:, 0:3],
                         start=True, stop=True)
        V.tensor_copy(FtPbS[:, 0:3], PSc[:, 0:3])

        for c in range(3):
            x = T9x[:, c * W:(c + 1) * W]    # fp16 scratch (blocked)
            y = T9y[:, c * W:(c + 1) * W]
            z = T9z[:, c * W:(c + 1) * W]
            TS(x, Uloc[:, 0:W], Fbc[:, 3 * c + 0:3 * c + 1], None,
               op0=OP.mult)
            TS(y, Uloc[:, W:2 * W], Fbc[:, 3 * c + 1:3 * c + 2], None,
               op0=OP.mult)
            A.mul(z, Uloc[:, 2 * W:3 * W], Fbc[:, 3 * c + 2:3 * c + 3])
            V.tensor_add(x, x, y)
            RS = 20
            V.tensor_add(_ap(Uord[:], c * W, [[12, RS], [3, 4], [1, 3]]),
                         _ap(x, 0, [[1, RS], [32, 4], [128, 3]]),
                         _ap(z, 0, [[1, RS], [32, 4], [128, 3]]))
            G.tensor_add(
                _ap(Uord[:], c * W + 12 * RS, [[12, 32 - RS], [3, 4], [1, 3]]),
                _ap(x, RS, [[1, 32 - RS], [32, 4], [128, 3]]),
                _ap(z, RS, [[1, 32 - RS], [32, 4], [128, 3]]))

        # masked scans seeded with Pinc -> final output directly (fp16)
        for c in range(3):
            V.tensor_tensor_scan(
                Pall[:, c * W:(c + 1) * W], Uord[:, c * W:(c + 1) * W],
                maskp[:], FtPbS[:, c:c + 1], op0=OP.add, op1=OP.mult)
            (nc.sync if c != 1 else nc.gpsimd).dma_start(
                out[:, c * W:(c + 1) * W], Pall[:, c * W:(c + 1) * W])


def _prep_alpha(input):
    # alphaN[r]=psi[r-1], alphaCA[r]=omega[r-1] (0 at r=0), alphaC[r]=phi[r];
    # then mod-4 block permutation within each 128-residue quarter.
    phi, psi, om = input[:, 0], input[:, 1], input[:, 2]
    z1 = np.zeros((input.shape[0], 1), np.float32)
    aN = np.concatenate([z1, psi[:, :-1]], axis=1)
    aCA = np.concatenate([z1, om[:, :-1]], axis=1)
    alpha = np.stack([aN, aCA, phi], axis=1)          # [B, 3, 512]
    alpha = alpha.reshape(-1, 3, QN, NR)
    perm = np.arange(NR).reshape(NB, 4).T.reshape(-1)  # sigma^-1: col->r
    alpha = alpha[..., perm]                           # blocked columns
    return alpha.transpose(0, 2, 1, 3)                 # [B, QN, 3, NR]


def _shard_alpha(alpha, i):
    sl = slice(i * BPC, (i + 1) * BPC)
    return np.ascontiguousarray(
        alpha[sl].transpose(1, 0, 2, 3).reshape(QN * BPC, 3 * NR))


def _build_consts():
    p = np.arange(128)
    i = np.arange(128)
    c16 = np.zeros((128, C16_N), np.float16)
    for q in range(QN):
        c16[:, C16_SELBQ + q * 128:C16_SELBQ + (q + 1) * 128] = (
            (i[None, :] % 32) == (p[:, None] - 32 * q))
    c16[:, C16_ONESR:C16_ONESR + NR] = 1.0
    c32 = np.zeros((128, C32_N), np.float32)
    c32[:, C32_SELCUM:C32_SELCUM + 128] = (
        (p[:, None] % 32 == i[None, :] % 32)
        & (p[:, None] // 32 < i[None, :] // 32))
    c32[0:BPC, C32_SELBT:C32_SELBT + 128] = (
        i[None, :] % 32 == np.arange(BPC)[:, None])
    c32[:, C32_JPLANE:C32_JPLANE + W] = np.arange(W)[None, :]
    return c16, c32


def _get_nc():
    if "nc" not in _CACHE:
        _CACHE["nc"] = _build_graph()
    return _CACHE["nc"]


def kernel(input, param, angles_length, trace=False):
    input = np.ascontiguousarray(input, dtype=np.float32)
    param = np.ascontiguousarray(param, dtype=np.float32)
    angles_length = np.ascontiguousarray(angles_length, dtype=np.int32)
    nc = _get_nc()
    alpha = _prep_alpha(input)
    if "consts" not in _CACHE:
        _CACHE["consts"] = _build_consts()
    c16, c32 = _CACHE["consts"]
    in_maps = []
    for i in range(NCORES):
        sl = slice(i * BPC, (i + 1) * BPC)
        in_maps.append({
            "input": _shard_alpha(alpha, i),
            "param": param,
            "angles_length": angles_length[sl],
            "c16": c16,
            "c32": c32,
        })
    res = run_bass_kernel_spmd(nc, in_maps, core_ids=list(range(NCORES)),
                               trace=trace)
    kernel._last_res = res
    outs = []
    for i in range(NCORES):
        r = res.results[i]["out"]          # [(q,b), (c,j)]
        r = r.reshape(QN, BPC, 3, W)
        r = np.transpose(r, (1, 0, 3, 2)).reshape(BPC, 3 * QN * W)
        outs.append(r)
    full = np.concatenate(outs, axis=0).astype(np.float32)
    if trace:
        kernel._last_exec_ns = res.exec_time_ns
    return full


kernel._last_exec_ns = None



# revision 33
# speedup vs baseline: 1.0162x; 1.0104x over previous
"""Trainium2 Bass kernel for Angles2Backbone (NeRF chain forward).

Full inputs: input [256,3,512] f32, param [6] f32, angles_length [256] i32.
Output: [256, 4608] f32  (coords of 1536 backbone atoms x 3, masked).

Sharding: pure data parallel over batch - 32 proteins per core x 8 cores.

Per-core algorithm (v3: fp16 scan machinery, mod-4 blocked residue layout):
  - 128 partitions = (quarter q)*32 + protein b; each row owns 128 residues.
  - Residue r of a quarter lives at column sigma(r) = 32*(r%4) + r//4
    (host-side permutation), so every pair/quad/expansion op reads and
    writes stride-1 column blocks -> DVE 2x fp16 mode throughout.
  - Trig via sin LUT at a/8, a/4 + double-angle chains (fp16).
  - Pre-pass builds per-residue rotation Rres (fp16) from scalar-folded
    bilinear terms; v-vectors (per-atom translations) in fp16.
  - pair (mod-4 blocks) -> P2, quad -> P4, Hillis-Steele over 32 quads.
  - Vector expansions Wodd/W2 written straight into a fused source tensor
    US so the superquad expansion is 5 big ops against a replicated,
    shifted prefix tensor L4.
  - Positions: blocked u -> reorder copy -> masked tensor_tensor_scan
    seeded with F^T.Pinc (cross-quarter fixup folded into the scan),
    then frame rotation F and store.
  - Cross-partition moves via PE matmuls only (no SBUF-SBUF DMA).
"""

import sys

sys.path.insert(0, "/opt/trn_rl_repo")

import numpy as np
import concourse.bass as bass
import concourse.bacc as bacc
import concourse.mybir as mybir
from concourse import tile
from concourse.bass_utils import run_bass_kernel_spmd

F32 = mybir.dt.float32
F16 = mybir.dt.float16
I32 = mybir.dt.int32
AF = mybir.ActivationFunctionType
OP = mybir.AluOpType

NCORES = 8
BPC = 32          # proteins per core
L = 512           # residues per protein
QN = 4            # chain quarters per protein (partition groups)
W = 384           # atoms per quarter
NR = 128          # residues per quarter
NB = 32           # columns per mod-4 block
PI = float(np.pi)

_CACHE = {}


# const block layouts (per partition)
C16_SELBQ = 0          # [512] selbq one-hots (fp16)
C16_L4 = 512           # [1152] L4 zero-init (fp16)
C16_ONESR = 1664       # [128] ones (fp16)
C16_N = 1792
C32_SELCUM = 0         # [128] cumulative masked selector (f32)
C32_SELBT = 128        # [128] selbT rows 0:32 (f32)
C32_JPLANE = 256       # [384] atom index (f32)
C32_N = 640


def _build_graph():
    nc = bacc.Bacc("TRN2", target_bir_lowering=False, debug=False,
                   num_devices=NCORES)
    inp = nc.dram_tensor("input", [QN * BPC, 3 * NR], F32,
                         kind="ExternalInput").ap()
    par = nc.dram_tensor("param", [6], F32, kind="ExternalInput").ap()
    alen = nc.dram_tensor("angles_length", [BPC], I32,
                          kind="ExternalInput").ap()
    c16 = nc.dram_tensor("c16", [128, C16_N], F16,
                         kind="ExternalInput").ap()
    c32 = nc.dram_tensor("c32", [128, C32_N], F32,
                         kind="ExternalInput").ap()
    out = nc.dram_tensor("out", [QN * BPC, 3 * W], F16,
                         kind="ExternalOutput").ap()
    with tile.TileContext(nc) as tc:
        _emit(nc, tc, inp, par, alen, c16, c32, out)
    nc.compile()
    return nc


def _ap(base_ap, off, dims):
    return bass.AP(base_ap.tensor, base_ap.offset + off,
                   [list(base_ap.ap[0])] + [list(d) for d in dims])


def _emit(nc, tc, inp, par, alen, c16, c32, out):
    import contextlib
    ctx = contextlib.ExitStack()
    with ctx:
        main = ctx.enter_context(tc.tile_pool(name="main", bufs=1))
        psum = ctx.enter_context(tc.tile_pool(name="psum", bufs=1,
                                              space="PSUM"))

        # ---------------- tiles ----------------
        alpha = main.tile([128, 3 * NR], F32, tag="alpha")
        ca = main.tile([128, 3 * NR], F16, tag="ca")
        sa = main.tile([128, 3 * NR], F16, tag="sa")
        # trig scratch (magic-number range reduction)
        tsq = main.tile([128, 3 * NR], F32, tag="tsq")   # squares scratch
        ts2 = main.tile([128, 3 * NR], F32, tag="ts2")   # scratch
        tmagic = main.tile([128, 3 * NR], F32, tag="tmagic")
        thalf = main.tile([128, 3 * NR], F32, tag="thalf")

        PP = main.tile([128, 4 * NR], F16, tag="PP")
        C1 = main.tile([128, 9 * NR], F16, tag="C1")
        QQ = main.tile([128, 6 * NR], F16, tag="QQ")
        RA = main.tile([128, 9 * NR], F16, tag="RA")     # Rres fp16
        vm = main.tile([128, 9 * NR], F16, tag="vm")
        P2 = main.tile([128, 9 * 2 * NB], F16, tag="P2")
        P2e2 = main.tile([128, 9 * 2 * NB], F16, tag="P2e2")
        P4A = main.tile([128, 9 * NB], F16, tag="P4A")
        P4B = main.tile([128, 9 * NB], F16, tag="P4B")
        US = main.tile([128, 9 * 4 * NB], F16, tag="US")
        WS = main.tile([128, 9 * 2 * NB], F16, tag="WS")
        T9a = main.tile([128, 9 * 4 * NB], F16, tag="T9a")
        T9b = main.tile([128, 9 * 4 * NB], F16, tag="T9b")
        T9c = main.tile([128, 9 * 4 * NB], F16, tag="T9c")
        T9w1 = main.tile([128, 9 * 2 * NB], F16, tag="T9w1")
        T9w2 = main.tile([128, 9 * 2 * NB], F16, tag="T9w2")
        T9w3 = main.tile([128, 9 * 2 * NB], F16, tag="T9w3")
        T9x = main.tile([128, 3 * W], F16, tag="T9x")
        T9y = main.tile([128, 3 * W], F16, tag="T9y")
        T9z = main.tile([128, 3 * W], F16, tag="T9z")
        T9af = main.tile([128, 18 * NB], F32, tag="T9af")
        T9bf = main.tile([128, 18 * NB], F32, tag="T9bf")
        T9cf = main.tile([128, 18 * NB], F32, tag="T9cf")
        Uloc = main.tile([128, 3 * W], F16, tag="Uloc")  # blocked u
        Uord = main.tile([128, 3 * W], F16, tag="Uord")  # ordered u
        Pall = main.tile([128, 3 * W], F16, tag="Pall")  # scanned+seeded out

        maskp = main.tile([128, W], F16, tag="maskp")
        thr = main.tile([128, 1], F32, tag="thr")
        Lsb = main.tile([BPC, 1], I32, tag="Lsb")
        Lf = main.tile([BPC, 1], F32, tag="Lf")
        Psb = main.tile([1, 6], F32, tag="Psb")
        kv = main.tile([1, 3], F32, tag="kv")
        Rv = main.tile([1, 3], F32, tag="Rv")
        NSC = 24
        vecs = main.tile([1, NSC], F32, tag="vecs")
        Vb = main.tile([128, NSC], F32, tag="Vb")
        zb1 = main.tile([1, 1], F32, tag="zb1")
        zb128 = main.tile([128, 1], F32, tag="zb128")
        warm = main.tile([1, 1], F32, tag="warm")

        # DMA'd constants (selbq one-hots, L4 zero init, ones, SELcum,
        # selbT, jplane)
        CON16 = main.tile([128, C16_N], F16, tag="CON16")
        CON32 = main.tile([128, C32_N], F32, tag="CON32")
        L4v = CON16[:, C16_L4:C16_L4 + 9 * 4 * NB]
        onesr = CON16[:, C16_ONESR:C16_ONESR + NR]
        SELcum = CON32[:, C32_SELCUM:C32_SELCUM + 128]
        selbTv = CON32[0:BPC, C32_SELBT:C32_SELBT + 128]
        jplane = CON32[:, C32_JPLANE:C32_JPLANE + W]

        # cross-quarter fixup (redundantly on all 128 rows, f32)
        Estack = main.tile([128, 36], F32, tag="Estack")
        Fstack = main.tile([128, 27], F32, tag="Fstack")
        Fbc = main.tile([128, 9], F32, tag="Fbc")
        mt0 = main.tile([128, 9], F32, tag="mt0")
        mt1 = main.tile([128, 9], F32, tag="mt1")
        # tail: local sums -> world quarter increments
        Tloc = main.tile([128, 3], F32, tag="Tloc")
        S9 = main.tile([128, 9], F32, tag="S9")
        S3 = main.tile([128, 3], F32, tag="S3")
        FtPbS = main.tile([128, 3], F32, tag="FtPbS")

        PSg = psum.tile([128, 36], F32, tag="PSg")
        PSc = psum.tile([128, 3], F32, tag="PSc")
        PSL = psum.tile([128, 1], F32, tag="PSL")

        V = nc.vector
        G = nc.gpsimd
        A = nc.scalar
        STT = nc.vector.scalar_tensor_tensor
        TS = nc.vector.tensor_scalar
        GTS = nc.gpsimd.tensor_scalar

        # ============ Phase A: DMAs + ACT warmup + setup ============
        nc.sync.dma_start(alpha[:], inp[:])
        nc.sync.dma_start(Psb[:], par[:])
        nc.sync.dma_start(Lsb[:], alen[:])
        nc.gpsimd.dma_start(CON16[:], c16[:])
        nc.gpsimd.dma_start(CON32[:], c32[:])
        V.memset(zb1[:], 0.0)
        V.memset(zb128[:], 0.0)
        # trigger the Sin table load immediately (Copy set loads after sins)
        A.activation(warm[:], zb1[:], AF.Sin, bias=zb1[:])

        # ============ Phase B: trig (2^23 magic range reduction + Sin) ====
        # ar = alpha - 2pi*round(alpha/2pi) in [-pi, pi]; sa = sin(ar);
        # ca = 1 - 2*sin(ar/2)^2. 1.5*2^23 keeps the rounding add in the
        # [2^23, 2^24) binade (ulp=1) for negative args too.
        # Per type block (N, CA, C) to pipeline the serial chain. Emitted
        # BEFORE the param-scalar block: DVE dispatch is in-order and the
        # param copies wait on the (later) param DMA sem.
        MAGIC = float(3 * 2 ** 22)

        def trig_dve(t):
            bs = slice(t * NR, (t + 1) * NR)
            V.tensor_scalar(tmagic[:, bs], alpha[:, bs], 1.0 / (2 * PI),
                            MAGIC, op0=OP.mult, op1=OP.add)
            V.tensor_scalar(tmagic[:, bs], tmagic[:, bs], MAGIC, None,
                            op0=OP.subtract)
            STT(ts2[:, bs], tmagic[:, bs], -2 * PI, alpha[:, bs],
                op0=OP.mult, op1=OP.add)

        def trig_sin(t):
            bs = slice(t * NR, (t + 1) * NR)
            A.activation(sa[:, bs], ts2[:, bs], AF.Sin, bias=zb128[:])
            A.activation(thalf[:, bs], ts2[:, bs], AF.Sin, bias=zb128[:],
                         scale=0.5)

        def trig_cos(t):
            bs = slice(t * NR, (t + 1) * NR)
            G.tensor_mul(tsq[:, bs], thalf[:, bs], thalf[:, bs])
            G.tensor_scalar(ca[:, bs], tsq[:, bs], -2.0, 1.0,
                            op0=OP.mult, op1=OP.add)

        trig_dve(0)
        trig_sin(0)
        trig_dve(1)

        # param scalars (wait on the param DMA; emitted between trig blocks)
        for t, idx in enumerate((5, 1, 3)):   # kappa: CA_C_N, C_N_CA, N_CA_C
            V.tensor_copy(kv[0:1, t:t + 1], Psb[0:1, idx:idx + 1])
        for t, idx in enumerate((4, 0, 2)):   # R: R_C_N, R_N_CA, R_CA_C
            V.tensor_copy(Rv[0:1, t:t + 1], Psb[0:1, idx:idx + 1])
        sk3 = main.tile([1, 3], F32, tag="sk3")
        ck3 = main.tile([1, 3], F32, tag="ck3")
        kvr = main.tile([1, 3], F32, tag="kvr")
        A.activation(sk3[:], kv[0:1, 0:3], AF.Sin, bias=zb1[:])
        A.activation(kvr[:], kv[0:1, 0:3], AF.Sin, bias=zb1[:], scale=0.5)
        trig_sin(1)
        trig_cos(0)
        trig_dve(2)
        trig_sin(2)
        trig_cos(1)
        trig_cos(2)
        A.copy(warm[:], zb1[:])     # Copy-set LUT load after the last Sin

        V.tensor_mul(kvr[:], kvr[:], kvr[:])
        V.tensor_scalar(ck3[:], kvr[:], -2.0, 1.0, op0=OP.mult, op1=OP.add)

        # scalar slots in vecs[1, NSC]:
        # 0:ckN 1:skN 2:ckA 3:skA 4:ckC 5:skC
        # 6:ckNckA 7:ckNskA 8:skNckA 9:skNskA
        # 10:nckN 11:nckA 12:nckC 13:nskA 14:nckNskA 15:nskNckA
        # 16:RNckN 17:RNskN 18:RCA 19:RC
        def vc(i):
            return vecs[0:1, i:i + 1]

        # interleave ck/sk into slots 0..5
        V.tensor_copy(_ap(vecs[:], 0, [[2, 3]]), ck3[0:1, 0:3])
        V.tensor_copy(_ap(vecs[:], 1, [[2, 3]]), sk3[0:1, 0:3])
        # outer product (ckN,skN) x (ckA,skA) -> slots 6..9
        V.tensor_mul(_ap(vecs[:], 6, [[2, 2], [1, 2]]),
                     _ap(vecs[:], 0, [[1, 2], [0, 2]]),
                     _ap(vecs[:], 2, [[0, 2], [1, 2]]))
        # negations: 10..12 = -(ckN,ckA,ckC) ; 13 = -skA ; 14,15 = -(7,8)
        V.tensor_scalar(_ap(vecs[:], 10, [[1, 3]]),
                        _ap(vecs[:], 0, [[2, 3]]), -1.0, None, op0=OP.mult)
        V.tensor_scalar(vc(13), vc(3), -1.0, None, op0=OP.mult)
        V.tensor_scalar(_ap(vecs[:], 14, [[1, 2]]),
                        _ap(vecs[:], 7, [[1, 2]]), -1.0, None, op0=OP.mult)
        # 16,17 = RN * (ckN, skN) ; 18,19 = RCA, RC
        V.tensor_mul(_ap(vecs[:], 16, [[1, 2]]),
                     _ap(Rv[:], 0, [[0, 2]]), _ap(vecs[:], 0, [[1, 2]]))
        V.tensor_copy(_ap(vecs[:], 18, [[1, 2]]), Rv[0:1, 1:3])
        G.partition_broadcast(Vb[:], vecs[:])


        S = {}
        for i, nm in enumerate(("ckN", "skN", "ckA", "skA", "ckC", "skC",
                                "ckNckA", "ckNskA", "skNckA", "skNskA",
                                "nckN", "nckA", "nckC", "nskA",
                                "nckNskA", "nskNckA", "RNckN", "RNskN",
                                "RCA", "RC")):
            S[nm] = Vb[:, i:i + 1]

        # ============ Phase D: C1 = B_N @ B_CA (fp16) ============
        cN, sN = ca[:, 0:NR], sa[:, 0:NR]
        cA, sA = ca[:, NR:2 * NR], sa[:, NR:2 * NR]
        cC, sC = ca[:, 2 * NR:3 * NR], sa[:, 2 * NR:3 * NR]

        def blk(t, e, lo=0, hi=NR):
            return t[:, e * NR + lo:e * NR + hi]

        pp1 = PP[:, 0 * NR:1 * NR]
        pp2 = PP[:, 1 * NR:2 * NR]
        pp3 = PP[:, 2 * NR:3 * NR]
        pp4 = PP[:, 3 * NR:4 * NR]
        V.tensor_mul(pp1, cN, cA)
        V.tensor_mul(pp2, sN, sA)
        G.tensor_mul(pp3, cN, sA)
        G.tensor_mul(pp4, sN, cA)
        c1 = C1[:]
        TS(blk(c1, 0), cA, S["skNskA"], S["ckNckA"],
           op0=OP.mult, op1=OP.add)                       # C1_00
        TS(blk(c1, 1), cA, S["nskNckA"], S["ckNskA"],
           op0=OP.mult, op1=OP.add)                       # C1_01
        V.tensor_scalar_mul(blk(c1, 2), sA, S["skN"])     # C1_02
        x = blk(c1, 3)                                    # C1_10
        A.mul(x, cN, S["skNckA"])
        STT(x, pp1, S["nckNskA"], x, op0=OP.mult, op1=OP.add)
        STT(x, pp2, S["skA"], x, op0=OP.mult, op1=OP.add)
        x = blk(c1, 4)                                    # C1_11
        A.mul(x, cN, S["skNskA"])
        STT(x, pp1, S["ckNckA"], x, op0=OP.mult, op1=OP.add)
        STT(x, pp2, S["nckA"], x, op0=OP.mult, op1=OP.add)
        STT(blk(c1, 5), pp3, S["nckN"], pp4,
            op0=OP.mult, op1=OP.subtract)                 # C1_12
        x = blk(c1, 6)                                    # C1_20
        A.mul(x, sN, S["skNckA"])
        STT(x, pp4, S["nckNskA"], x, op0=OP.mult, op1=OP.add)
        STT(x, pp3, S["nskA"], x, op0=OP.mult, op1=OP.add)
        x = blk(c1, 7)                                    # C1_21
        A.mul(x, sN, S["skNskA"])
        STT(x, pp4, S["ckNckA"], x, op0=OP.mult, op1=OP.add)
        STT(x, pp3, S["ckA"], x, op0=OP.mult, op1=OP.add)
        STT(blk(c1, 8), pp2, S["nckN"], pp1,
            op0=OP.mult, op1=OP.add)                      # C1_22

        # residue-0 of q=0 rows: C1 := B_CA(0) (alpha_CA(0)=0)
        r0s = slice(0, BPC)
        o1 = CON16[r0s, C16_ONESR:C16_ONESR + 1]
        V.tensor_scalar_mul(c1[r0s, 0 * NR:0 * NR + 1], o1, S["ckA"][r0s])
        V.tensor_scalar_mul(c1[r0s, 1 * NR:1 * NR + 1], o1, S["skA"][r0s])
        V.memset(c1[r0s, 2 * NR:2 * NR + 1], 0.0)
        V.tensor_scalar_mul(c1[r0s, 3 * NR:3 * NR + 1], o1, S["skA"][r0s])
        V.tensor_scalar_mul(c1[r0s, 4 * NR:4 * NR + 1], o1, S["nckA"][r0s])
        V.memset(c1[r0s, 5 * NR:5 * NR + 1], 0.0)
        V.memset(c1[r0s, 6 * NR:6 * NR + 1], 0.0)
        V.memset(c1[r0s, 7 * NR:7 * NR + 1], 0.0)
        V.memset(c1[r0s, 8 * NR:8 * NR + 1], -1.0)

        # ============ Phase E: Rres = C1 @ B_C -> RA (fp16), vm ========
        ra = RA[:]
        for i in range(3):
            (V if i != 2 else G).tensor_mul(blk(QQ[:], i),
                                            blk(c1, 3 * i + 1), cC)
            (G if i != 2 else V).tensor_mul(blk(QQ[:], 3 + i),
                                            blk(c1, 3 * i + 2), sC)
        for i in range(3):
            q1i = blk(QQ[:], i)
            q2i = blk(QQ[:], 3 + i)
            qs = T9w3[:, i * NR:(i + 1) * NR]
            (V if i != 2 else G).tensor_add(qs, q1i, q2i)
            x = blk(ra, 3 * i + 0)
            A.mul(x, blk(c1, 3 * i + 0), S["ckC"])
            STT(x, qs, S["skC"], x, op0=OP.mult, op1=OP.add)
            x = blk(ra, 3 * i + 1)
            A.mul(x, blk(c1, 3 * i + 0), S["skC"])
            STT(x, qs, S["nckC"], x, op0=OP.mult, op1=OP.add)
            t1 = T9w1[:, i * NR:(i + 1) * NR]
            t2 = T9w2[:, i * NR:(i + 1) * NR]
            V.tensor_mul(t1, blk(c1, 3 * i + 1), sC)
            G.tensor_mul(t2, blk(c1, 3 * i + 2), cC)
            V.tensor_sub(blk(ra, 3 * i + 2), t1, t2)

        # v-vectors (fp16): vm plane p = 3*vec + coord
        vmv = vm[:]
        V.tensor_scalar_mul(blk(vmv, 0), onesr, S["RNckN"])
        A.mul(blk(vmv, 1), cN, S["RNskN"])
        A.mul(blk(vmv, 2), sN, S["RNskN"])
        for i in range(3):
            A.mul(blk(vmv, 3 + i), blk(c1, 3 * i + 0), S["RCA"])
            A.mul(blk(vmv, 6 + i), blk(ra, 3 * i + 0), S["RC"])
        # US g0 block = vm m0 (feeds the U expansion's first half)
        A.copy(_ap(US[:], 0, [[128, 9], [1, 32]]),
               _ap(vmv, 0, [[NR, 9], [1, 32]]))

        # ============ setup: mask (selectors come from const DMA) ========
        V.tensor_copy(Lf[:], Lsb[:])
        nc.tensor.matmul(PSL[:, 0:1], selbTv, Lf[:, 0:1],
                         start=True, stop=True)
        for q in range(QN):
            TS(thr[q * BPC:(q + 1) * BPC, 0:1],
               PSL[q * BPC:(q + 1) * BPC, 0:1],
               3.0, float(q * W), op0=OP.mult, op1=OP.subtract)
        TS(maskp[:], jplane, thr[:, 0:1], None, op0=OP.is_lt)

        # ============ Phase F: scan machinery (fp16) ============
        # generic fused 3-term matmul group, col-split DVE/Pool
        def fused(Lf_, Rf_, Of_, Tf_, n, split=0.85, ta=None, tb=None,
                  tcx=None, eng=None):
            ta = T9a if ta is None else ta
            tb = T9b if tb is None else tb
            tcx = (T9c if ta is T9a else T9cf) if tcx is None else tcx
            if eng is not None:
                segs = [(eng, 0, n)]
            else:
                cut = min(n, max(0, int(n * split)))
                segs = []
                if cut > 0:
                    segs.append((V, 0, cut))
                if cut < n:
                    segs.append((G, cut, n))
            for E, c0, c1_ in segs:
                E.tensor_mul(Tf_(ta, c0, c1_), Lf_(0, c0, c1_),
                             Rf_(0, c0, c1_))
                E.tensor_mul(Tf_(tb, c0, c1_), Lf_(1, c0, c1_),
                             Rf_(1, c0, c1_))
                E.tensor_mul(Tf_(tcx, c0, c1_), Lf_(2, c0, c1_),
                             Rf_(2, c0, c1_))
                E.tensor_add(Of_(c0, c1_), Tf_(ta, c0, c1_),
                             Tf_(tb, c0, c1_))
                E.tensor_add(Of_(c0, c1_), Of_(c0, c1_), Tf_(tcx, c0, c1_))

        # --- pair: P2[b, j] = RA[blk 2b, j] @ RA[blk 2b+1, j]
        for b in range(2):
            base = 64 * b

            def PL(k, c0, c1_, base=base):
                return _ap(ra, k * NR + base + c0,
                           [[3 * NR, 3], [0, 3], [1, c1_ - c0]])

            def PR(k, c0, c1_, base=base):
                return _ap(ra, 3 * k * NR + base + NB + c0,
                           [[0, 3], [NR, 3], [1, c1_ - c0]])

            def PO(c0, c1_, base=32 * b):
                return _ap(P2[:], base + c0,
                           [[192, 3], [64, 3], [1, c1_ - c0]])

            def PT(t, c0, c1_, base=288 * b):
                return _ap(t[:], base + c0, [[96, 3], [32, 3], [1, c1_ - c0]])

            fused(PL, PR, PO, PT, NB, eng=V)

        # --- Wodd emitters
        def emit_wodd(b, eng):
            dst, dstride = ((US, 128), (WS, 64))[b]
            base = 64 * b

            def WL(k, c0, c1_, base=base):
                return _ap(ra, k * NR + base + c0,
                           [[3 * NR, 3], [0, 3], [1, c1_ - c0]])

            def WR(k, c0, c1_, base=base):
                return _ap(vmv, k * NR + base + NB + c0,
                           [[0, 3], [3 * NR, 3], [1, c1_ - c0]])

            def WO(c0, c1_, dst=dst, ds=dstride):
                return _ap(dst[:], NB + c0,
                           [[ds, 3], [3 * ds, 3], [1, c1_ - c0]])

            def WT(t, c0, c1_, base=576 + 288 * b):
                return _ap(t[:], base + c0,
                           [[32, 3], [96, 3], [1, c1_ - c0]])

            if eng is None:
                fused(WL, WR, WO, WT, NB, split=0.3)
            else:
                fused(WL, WR, WO, WT, NB, eng=eng)

        # --- quad: P4[j] = P2[b0, j] @ P2[b1, j]
        def QL(k, c0, c1_):
            return _ap(P2[:], k * 64 + c0, [[192, 3], [0, 3], [1, c1_ - c0]])

        def QR(k, c0, c1_):
            return _ap(P2[:], 3 * k * 64 + NB + c0,
                       [[0, 3], [64, 3], [1, c1_ - c0]])

        def QO(c0, c1_):
            return _ap(P4A[:], c0, [[96, 3], [32, 3], [1, c1_ - c0]])

        def QT(t, c0, c1_):
            return _ap(t[:], c0, [[96, 3], [32, 3], [1, c1_ - c0]])

        fused(QL, QR, QO, QT, NB, split=0.84)

        # --- W2 merged: out US cols 64:128 = P2even @ WS
        def emit_w2():
            A.copy(_ap(P2e2[:], 0, [[64, 9], [32, 2], [1, 32]]),
                   _ap(P2[:], 0, [[64, 9], [0, 2], [1, 32]]))
            A.copy(_ap(WS[:], 0, [[64, 9], [1, 32]]),
                   _ap(vmv, 64, [[NR, 9], [1, 32]]))

            def W2L(k, c0, c1_):
                return _ap(P2e2[:], k * 64 + c0,
                           [[192, 3], [0, 3], [1, c1_ - c0]])

            def W2R(k, c0, c1_):
                return _ap(WS[:], k * 64 + c0,
                           [[0, 3], [192, 3], [1, c1_ - c0]])

            def W2O(c0, c1_):
                return _ap(US[:], 64 + c0,
                           [[128, 3], [384, 3], [1, c1_ - c0]])

            def W2T(t, c0, c1_):
                return _ap(t[:], c0, [[64, 3], [192, 3], [1, c1_ - c0]])

            fused(W2L, W2R, W2O, W2T, 2 * NB, split=0.9,
                  ta=T9w1, tb=T9w2, tcx=T9w3)

        # --- Hillis-Steele over 32 quads, expansions interleaved
        def hs_step(srcb, dstb, s):
            n = NB - s
            sv = srcb.rearrange("p (e j) -> p e j", e=9)
            dv = dstb.rearrange("p (e j) -> p e j", e=9)
            V.tensor_copy(dv[:, :, 0:s], sv[:, :, 0:s])

            def HL(k, c0, c1_):
                return _ap(srcb, k * NB + c0,
                           [[96, 3], [0, 3], [1, c1_ - c0]])

            def HR(k, c0, c1_):
                return _ap(srcb, 3 * k * NB + s + c0,
                           [[0, 3], [32, 3], [1, c1_ - c0]])

            def HO(c0, c1_):
                return _ap(dstb, s + c0, [[96, 3], [32, 3], [1, c1_ - c0]])

            def HT(t, c0, c1_):
                return _ap(t[:], c0, [[96, 3], [32, 3], [1, c1_ - c0]])

            fused(HL, HR, HO, HT, n, split=0.8)

        def emit_u(u0, u1):
            def UL(k, c0, c1_):
                return _ap(L4v, k * 128 + u0 + c0,
                           [[384, 3], [0, 3], [1, c1_ - c0]])

            def UR(k, c0, c1_):
                return _ap(US[:], k * 128 + u0 + c0,
                           [[0, 3], [384, 3], [1, c1_ - c0]])

            def UO(c0, c1_):
                return _ap(Uloc[:], u0 + c0,
                           [[384, 3], [128, 3], [1, c1_ - c0]])

            def UT(t, c0, c1_):
                return _ap(t[:], u0 + c0,
                           [[384, 3], [128, 3], [1, c1_ - c0]])

            fused(UL, UR, UO, UT, u1 - u0, split=0.8)

        def emit_ub(u0, u1):
            def UL(k, c0, c1_):
                return _ap(L4v, k * 128 + u0 + c0,
                           [[384, 3], [0, 3], [1, c1_ - c0]])

            def UR(k, c0, c1_):
                return _ap(US[:], k * 128 + u0 + c0,
                           [[0, 3], [384, 3], [1, c1_ - c0]])

            def UO(c0, c1_):
                return _ap(Uloc[:], u0 + c0,
                           [[384, 3], [128, 3], [1, c1_ - c0]])

            def UT(t, c0, c1_):
                return _ap(t[:], u0 + c0,
                           [[384, 3], [128, 3], [1, c1_ - c0]])

            fused(UL, UR, UO, UT, u1 - u0, split=0.9)

        bufs = [P4A, P4B]
        emit_wodd(0, None)
        hs_step(bufs[0][:], bufs[1][:], 1)
        hs_step(bufs[1][:], bufs[0][:], 2)
        hs_step(bufs[0][:], bufs[1][:], 4)
        hs_step(bufs[1][:], bufs[0][:], 8)
        # L4 prefix cols 0:16 are final after HS4 -> copy during HS5 (DVE)
        V.tensor_copy(_ap(L4v, 1, [[128, 9], [32, 4], [1, 16]]),
                      _ap(bufs[0][:], 0, [[NB, 9], [0, 4], [1, 16]]))
        hs_step(bufs[0][:], bufs[1][:], 16)
        emit_wodd(1, V)
        Rscan = bufs[1][:]    # P4B
        V.tensor_copy(_ap(L4v, 17, [[128, 9], [32, 4], [1, NB - 17]]),
                      _ap(Rscan, 16, [[NB, 9], [0, 4], [1, NB - 17]]))
        emit_u(0, 2 * NB)       # g0/g1 half (needs only Wodd b0 + vm)
        emit_w2()

        # --- rotation fixup: E gathered to ALL rows, F chain, slice Fbc ---
        for q in range(QN):
            nc.tensor.matmul(
                PSg[:, q * 9:(q + 1) * 9],
                CON16[:, C16_SELBQ + q * 128:C16_SELBQ + (q + 1) * 128],
                _ap(Rscan, NB - 1, [[NB, 9]]), start=True, stop=True)
        V.tensor_copy(Estack[:, 0:36], PSg[:, 0:36])
        V.tensor_copy(Fstack[:, 0:9], Estack[:, 0:9])
        fs = Fstack[:]
        es = Estack[:]

        def ap2(base_ap, off, dims):
            return _ap(base_ap, off, dims)

        for q in (1, 2):
            FL = lambda k: ap2(fs, (q - 1) * 9 + k, [[3, 3], [0, 3]])
            ER = lambda k: ap2(es, q * 9 + 3 * k, [[0, 3], [1, 3]])
            MT = lambda t: ap2(t[:], 0, [[3, 3], [1, 3]])
            FO = ap2(fs, q * 9, [[3, 3], [1, 3]])
            V.tensor_mul(MT(mt0), FL(0), ER(0))
            V.tensor_mul(MT(mt1), FL(1), ER(1))
            V.tensor_add(MT(mt0), MT(mt0), MT(mt1))
            V.tensor_mul(MT(mt1), FL(2), ER(2))
            V.tensor_add(FO, MT(mt0), MT(mt1))
        # Fbc: rows 0:32 identity; quarter q rows take F_q slice
        V.memset(Fbc[0:BPC, 0:9], 0.0)
        V.memset(bass.AP(Fbc[:].tensor, Fbc[:].offset,
                         [[Fbc[:].ap[0][0], BPC], [4, 3]]), 1.0)
        for q in (1, 2, 3):
            V.tensor_copy(Fbc[q * BPC:(q + 1) * BPC, 0:9],
                          Fstack[q * BPC:(q + 1) * BPC,
                                 (q - 1) * 9:q * 9])

        # --- U merged: Uloc[c][j*128 + g*32 + r] = L4 @ US
        emit_ub(2 * NB, 4 * NB)  # g2/g3 half (needs W2), DVE only

        # seeds: r=0 of each g-block: identity prefix -> u = US col g*32
        V.tensor_copy(_ap(Uloc[:], 0, [[384, 3], [128, 3], [32, 4]]),
                      _ap(US[:], 0, [[128, 3], [384, 3], [32, 4]]))
        # atom 0 of whole chain (q=0 rows) at origin
        V.memset(bass.AP(Uloc[:].tensor, Uloc[:].offset,
                         [[Uloc[:].ap[0][0], BPC], [384, 3]]), 0.0)

        # ============ Phase G: tail ============
        # F-apply fused with blocked->ordered reorder:
        #   Uord_c[a] = F_c0*Ux[b(a)] + F_c1*Uy[b(a)] + F_c2*Uz[b(a)]
        # (walk order (r,g,j): out stride-1-ish ordered, ins blocked)
        def ordw(t, c):
            return _ap(t[:], c * W, [[12, 32], [3, 4], [1, 3]])

        def blkw(t, c):
            return _ap(t[:], c * W, [[1, 32], [32, 4], [128, 3]])

        # --- quarter increments BEFORE the scans (from local sums):
        #   Tloc_c = sum over row of Uloc plane c (full quarter; a partial
        #   quarter's sum only feeds fully-masked later quarters)
        #   S = Fbc @ Tloc (world frame), Pinc = SELcum-gather of S.
        for c in range(3):
            V.tensor_scalar(T9a[:, c * W:(c + 1) * W],
                            Uloc[:, c * W:(c + 1) * W], 1.0, 0.0,
                            op0=OP.mult, op1=OP.add,
                            accum_out=Tloc[:, c:c + 1])
        V.tensor_mul(S9[:, 0:9], Fbc[:, 0:9],
                     _ap(Tloc[:], 0, [[0, 3], [1, 3]]))
        V.tensor_add(S3[:, 0:3], _ap(S9[:], 0, [[3, 3]]),
                     _ap(S9[:], 1, [[3, 3]]))
        V.tensor_add(S3[:, 0:3], S3[:, 0:3], _ap(S9[:], 2, [[3, 3]]))
        nc.tensor.matmul(PSc[:, 0:3], SELcum, S3[:, 0:3],
                         start=True, stop=True)
        V.tensor_copy(FtPbS[:, 0:3], PSc[:, 0:3])

        for c in range(3):
            x = T9x[:, c * W:(c + 1) * W]    # fp16 scratch (blocked)
            y = T9y[:, c * W:(c + 1) * W]
            z = T9z[:, c * W:(c + 1) * W]
            TS(x, Uloc[:, 0:W], Fbc[:, 3 * c + 0:3 * c + 1], None,
               op0=OP.mult)
            TS(y, Uloc[:, W:2 * W], Fbc[:, 3 * c + 1:3 * c + 2], None,
               op0=OP.mult)
            A.mul(z, Uloc[:, 2 * W:3 * W], Fbc[:, 3 * c + 2:3 * c + 3])
            V.tensor_add(x, x, y)
            RS = 20
            V.tensor_add(_ap(Uord[:], c * W, [[12, RS], [3, 4], [1, 3]]),
                         _ap(x, 0, [[1, RS], [32, 4], [128, 3]]),
                         _ap(z, 0, [[1, RS], [32, 4], [128, 3]]))
            G.tensor_add(
                _ap(Uord[:], c * W + 12 * RS, [[12, 32 - RS], [3, 4], [1, 3]]),
                _ap(x, RS, [[1, 32 - RS], [32, 4], [128, 3]]),
                _ap(z, RS, [[1, 32 - RS], [32, 4], [128, 3]]))

        # masked scans seeded with Pinc -> final output directly (fp16)
        for c in range(3):
            V.tensor_tensor_scan(
                Pall[:, c * W:(c + 1) * W], Uord[:, c * W:(c + 1) * W],
                maskp[:], FtPbS[:, c:c + 1], op0=OP.add, op1=OP.mult)
            (nc.sync if c != 1 else nc.gpsimd).dma_start(
                out[:, c * W:(c + 1) * W], Pall[:, c * W:(c + 1) * W])


def _prep_alpha(input):
    # alphaN[r]=psi[r-1], alphaCA[r]=omega[r-1] (0 at r=0), alphaC[r]=phi[r];
    # then mod-4 block permutation within each 128-residue quarter.
    phi, psi, om = input[:, 0], input[:, 1], input[:, 2]
    z1 = np.zeros((input.shape[0], 1), np.float32)
    aN = np.concatenate([z1, psi[:, :-1]], axis=1)
    aCA = np.concatenate([z1, om[:, :-1]], axis=1)
    alpha = np.stack([aN, aCA, phi], axis=1)          # [B, 3, 512]
    alpha = alpha.reshape(-1, 3, QN, NR)
    perm = np.arange(NR).reshape(NB, 4).T.reshape(-1)  # sigma^-1: col->r
    alpha = alpha[..., perm]                           # blocked columns
    return alpha.transpose(0, 2, 1, 3)                 # [B, QN, 3, NR]


def _shard_alpha(alpha, i):
    sl = slice(i * BPC, (i + 1) * BPC)
    return np.ascontiguousarray(
        alpha[sl].transpose(1, 0, 2, 3).reshape(QN * BPC, 3 * NR))


def _build_consts():
    p = np.arange(128)
    i = np.arange(128)
    c16 = np.zeros((128, C16_N), np.float16)
    for q in range(QN):
        c16[:, C16_SELBQ + q * 128:C16_SELBQ + (q + 1) * 128] = (
            (i[None, :] % 32) == (p[:, None] - 32 * q))
    c16[:, C16_ONESR:C16_ONESR + NR] = 1.0
    c32 = np.zeros((128, C32_N), np.float32)
    c32[:, C32_SELCUM:C32_SELCUM + 128] = (
        (p[:, None] % 32 == i[None, :] % 32)
        & (p[:, None] // 32 < i[None, :] // 32))
    c32[0:BPC, C32_SELBT:C32_SELBT + 128] = (
        i[None, :] % 32 == np.arange(BPC)[:, None])
    c32[:, C32_JPLANE:C32_JPLANE + W] = np.arange(W)[None, :]
    return c16, c32


def _get_nc():
    if "nc" not in _CACHE:
        _CACHE["nc"] = _build_graph()
    return _CACHE["nc"]


def kernel(input, param, angles_length, trace=False):
    input = np.ascontiguousarray(input, dtype=np.float32)
    param = np.ascontiguousarray(param, dtype=np.float32)
    angles_length = np.ascontiguousarray(angles_length, dtype=np.int32)
    nc = _get_nc()
    alpha = _prep_alpha(input)
    if "consts" not in _CACHE:
        _CACHE["consts"] = _build_consts()
    c16, c32 = _CACHE["consts"]
    in_maps = []
    for i in range(NCORES):
        sl = slice(i * BPC, (i + 1) * BPC)
        in_maps.append({
            "input": _shard_alpha(alpha, i),
            "param": param,
            "angles_length": angles_length[sl],
            "c16": c16,
            "c32": c32,
        })
    res = run_bass_kernel_spmd(nc, in_maps, core_ids=list(range(NCORES)),
                               trace=trace)
    kernel._last_res = res
    outs = []
    for i in range(NCORES):
        r = res.results[i]["out"]          # [(q,b), (c,j)]
        r = r.reshape(QN, BPC, 3, W)
        r = np.transpose(r, (1, 0, 3, 2)).reshape(BPC, 3 * QN * W)
        outs.append(r)
    full = np.concatenate(outs, axis=0).astype(np.float32)
    if trace:
        kernel._last_exec_ns = res.exec_time_ns
    return full


kernel._last_exec_ns = None

